# revision 42
# baseline (speedup 1.0000x reference)
"""Trainium2 Bass kernel for nn_AttentionBlock (B=8, H=W=32, C=512, 8 heads).

Strategy: data-parallel over batch -- each of the 8 NeuronCores processes one
batch element end-to-end (no collectives).  Per core:

  x [T=1024, C=512] -> qkv -> per-head attention (T x T softmax) -> out proj.

v2 design (ACT-bound pipeline):
  * all matmul data is bf16 (host pre-casts); PSUM accumulation stays fp32.
  * S^T = k^T q is computed with K=64 ROW-TILED matmul pairs: head 2p on PE
    row-groups 0-1 (SBUF partitions 0:64), head 2p+1 on row-groups 2-3
    (partitions 64:128).  The two matmuls run concurrently in the array, so
    a head-pair s-tile costs ~2x512 columns instead of 4x512.
  * softmax exp runs on ScalarE (the only exp engine) in N=1024 calls and is
    the phase-2 bottleneck (~73us); everything else (QKV projection, PV,
    out-projection) is woven into the PE gaps between exp calls.
  * denominators come free from a ones-column appended to V (PV row 64);
    normalization = DVE multiply by a GPSIMD-broadcast reciprocal.
  * out-projection uses anorm tiles as lhsT so the output lands directly in
    [t, c] layout -- no output transpose.  Output is bf16; host casts back.
  * no max-subtraction: logits are ~N(0,1) by construction (1/8 scale is
    folded into the ScalarE exp activation).
"""

import math
import os
from contextlib import ExitStack

import numpy as np

import concourse.bass as bass
import concourse.mybir as mybir
import concourse.tile as tile
from concourse import bacc

T = 1024          # tokens per batch element (32*32)
C = 512           # channels
HEADS = 8
HC = C // HEADS   # 64
P = 128           # partitions
NT = T // P       # 8 t-tiles (also 8 s-tiles)
NCT = C // P      # 4 c-tiles
CHUNK = 512       # matmul moving-operand chunk (one fp32 PSUM bank)
NCH = T // CHUNK  # 2 chunks
NPAIR = HEADS // 2
F32 = mybir.dt.float32
BF16 = mybir.dt.bfloat16
EXP_SCALE = 1.0 / math.sqrt(HC)  # (1/sqrt(sqrt(hc)))^2 applied to q.k
VSTRIDE = HC + 1  # 65: v columns + ones column per head
VAW = HEADS * VSTRIDE + (P - VSTRIDE)  # PV lhsT 128-wide reads stay in-tile


def build_program(debug_dumps=False):
    nc = bacc.Bacc("TRN2", num_devices=8, debug=False)

    x_d = nc.dram_tensor("x", [T, C], BF16, kind="ExternalInput")
    wqkv_d = nc.dram_tensor("qkv_w", [C, 3 * C], BF16, kind="ExternalInput")
    wout_d = nc.dram_tensor("out_w", [C, C], BF16, kind="ExternalInput")
    qkb_d = nc.dram_tensor("qk_b", [2 * C], F32, kind="ExternalInput")
    ob_d = nc.dram_tensor("out_b", [C], F32, kind="ExternalInput")
    out_d = nc.dram_tensor("out", [T, C], BF16, kind="ExternalOutput")
    dbg = {}
    if debug_dumps:
        for nm, shp in [
            ("dbg_xT", [P, NCT * T]), ("dbg_q0", [P, T]), ("dbg_k0", [P, T]),
            ("dbg_exh0", [P, 2 * T]), ("dbg_va0", [P, VAW]),
            ("dbg_an0", [P, T]),
        ]:
            dbg[nm] = nc.dram_tensor(nm, shp, BF16, kind="ExternalOutput")

    with tile.TileContext(nc) as tc, ExitStack() as ctx:
        from concourse.masks import make_identity

        # ---------------- SBUF pools ----------------
        const = ctx.enter_context(tc.tile_pool(name="const", bufs=1))
        persist = ctx.enter_context(tc.tile_pool(name="persist", bufs=1))
        workp = ctx.enter_context(tc.tile_pool(name="workp", bufs=1))

        # x in FOUR batched DMAs (2 per HWDGE queue) so the first quarter
        # lands ASAP; per-DMA issue cost is ~0.6us.  Layout:
        # x_in[:, i*C + c] = x[i*128 + p, c] (t-tile-major).
        x_in = persist.tile([P, NT * C], BF16, tag="x_in", name="x_in")
        xr = x_d.ap().rearrange("(i p) c -> p i c", p=P)  # [128, 8, 512]
        xv = x_in[:].rearrange("p (i c) -> p i c", i=NT)
        nc.sync.dma_start(xv[:, 0:2, :], xr[:, 0:2, :])
        nc.scalar.dma_start(xv[:, 2:4, :], xr[:, 2:4, :])
        nc.sync.dma_start(xv[:, 4:6, :], xr[:, 4:6, :])
        nc.scalar.dma_start(xv[:, 6:8, :], xr[:, 6:8, :])

        identity = const.tile([P, P], BF16, tag="ident", name="ident")
        make_identity(nc, identity[:])
        warm_rhs = const.tile([P, CHUNK], BF16, tag="warm", name="warm_rhs")
        nc.gpsimd.memset(warm_rhs[:], 0.0)
        # ones1: K=1 all-ones lhsT for the tail's PE-broadcast normalize
        ones1 = const.tile([1, P], F32, tag="ones1", name="ones1")
        nc.gpsimd.memset(ones1[:], 1.0)

        # qkv weights: q/k columns first (they gate the qk projections),
        # v columns on the slower SWDGE queue afterwards
        wq = []  # [c-tile][128, 1536] bf16
        for m in range(NCT):
            t_ = persist.tile([P, 3 * C], BF16, tag=f"wq{m}", name=f"wq{m}")
            eng = nc.sync if m % 2 == 0 else nc.scalar
            eng.dma_start(t_[:, 0:2 * C],
                          wqkv_d.ap()[m * P:(m + 1) * P, 0:2 * C])
            wq.append(t_)
        for m in range(NCT):
            nc.gpsimd.dma_start(wq[m][:, 2 * C:3 * C],
                                wqkv_d.ap()[m * P:(m + 1) * P, 2 * C:3 * C])

        # bias tiles; column m = qk_b[128m:128m+128]
        qkb_all = const.tile([P, 2 * C // P], F32, tag="qkball", name="qkb_all")
        nc.gpsimd.dma_start(
            qkb_all[:], qkb_d.ap().rearrange("(m p) -> p m", p=P)
        )
        qkb_t = [qkb_all[:, m:m + 1] for m in range(2 * C // P)]
        ob_row = const.tile([1, C], F32, tag="obrow", name="ob_row")
        nc.gpsimd.dma_start(ob_row[:], ob_d.ap().rearrange("(o c) -> o c", o=1))
        ob_bcast = const.tile([P, C], F32, tag="obb", name="ob_bcast")
        nc.gpsimd.partition_broadcast(ob_bcast[:], ob_row[:], channels=P)

        # out-proj weights: single batched DMA, needed only in phase 3
        wo_all = persist.tile([P, NCT * C], BF16, tag="wo", name="wo_all")
        nc.gpsimd.dma_start(
            wo_all[:].rearrange("p (m c) -> p m c", m=NCT),
            wout_d.ap().rearrange("(m p) c -> p m c", p=P),
        )
        wo = [wo_all[:, m * C:(m + 1) * C] for m in range(NCT)]

        # persistent activation tiles
        # xT_all[:, cc*T + t] = x^T tile cc: [c-within-tile, t]
        xT_all = persist.tile([P, NCT * T], BF16, tag="xT", name="xT_all")
        qkT = [persist.tile([P, T], BF16, tag=f"qk{m}", name=f"qk{m}")
               for m in range(NCT)]
        # kT2[m]: rows 0:64 = k^T head 2m, rows 64:128 = k^T head 2m+1
        kT2 = [persist.tile([P, T], BF16, tag=f"k2{m}", name=f"k2{m}")
               for m in range(NCT)]
        vaug = [persist.tile([P, VAW], BF16, tag=f"va{i}", name=f"va{i}")
                for i in range(NT)]
        for i in range(NT):
            nc.gpsimd.memset(vaug[i][:], 1.0)  # ones column (+padding) preset
        anorm = [persist.tile([P, T], BF16, tag=f"an{m}", name=f"an{m}")
                 for m in range(NCT)]

        # ================= prologue =================
        # HAM warm-up: real (non-transpose) matmuls on junk data while the x
        # DMA lands, so the PE clock is at 8/8 when the real work starts.
        with tc.tile_pool(name="ps_warm", bufs=1, space="PSUM") as ps_warm:
            ps_w = ps_warm.tile([P, CHUNK], F32, tag="w", name="ps_w")
            for _ in range(5):
                nc.tensor.matmul(ps_w[:], identity[:], warm_rhs[:],
                                 start=True, stop=True)

        # exp ACT-table preload: a tiny dummy exp during the DMA wait pays the
        # ~2.7us one-time table-load cost before the real pipeline needs it.
        scratch16 = workp.tile([1, 16], F32, tag="scr16", name="scratch16")
        nc.scalar.activation(
            scratch16[:], warm_rhs[0:1, 0:16],
            mybir.ActivationFunctionType.Exp, scale=1.0)

        qk_cm = tc.tile_pool(name="ps_qk", bufs=2, space="PSUM")
        qk_pool = qk_cm.__enter__()

        tr_cm = tc.tile_pool(name="ps_tr", bufs=2, space="PSUM", side="right")
        tr_pool = tr_cm.__enter__()

        def emit_transpose(i):
            # x PE transpose; xT_all[:, cc*T + i*128 : ...] gets tile (i, cc)
            ps_tr = tr_pool.tile([P, C], BF16, tag="tr", name="ps_tr")
            for cc in range(NCT):
                nc.tensor.transpose(
                    ps_tr[:, cc * P:(cc + 1) * P],
                    x_in[:, i * C + cc * P: i * C + (cc + 1) * P],
                    identity[:],
                )
            # one strided copy: dest [128, cc, 128] with cc-stride T
            dst = xT_all[:].rearrange("p (cc t) -> p cc t", cc=NCT)
            nc.vector.tensor_copy(
                dst[:, :, i * P:(i + 1) * P],
                ps_tr[:].rearrange("p (cc q) -> p cc q", cc=NCT),
            )

        def emit_qk(m, j):
            ps_qk = qk_pool.tile([P, CHUNK], F32, tag="qk", name="ps_qk")
            js = slice(j * CHUNK, (j + 1) * CHUNK)
            for cc in range(NCT):
                nc.tensor.matmul(
                    ps_qk[:],
                    wq[cc][:, m * P:(m + 1) * P],
                    xT_all[:, cc * T + j * CHUNK: cc * T + (j + 1) * CHUNK],
                    start=(cc == 0),
                    stop=(cc == NCT - 1),
                )
            dstt = qkT[m] if m < NCT else kT2[m - NCT]
            nc.vector.tensor_scalar_add(dstt[:, js], ps_qk[:], qkb_t[m][:])

        def emit_v(i):
            ps_v = qk_pool.tile([P, CHUNK], F32, tag="qk", name="ps_v")
            for cc in range(NCT):
                nc.tensor.matmul(
                    ps_v[:],
                    xT_all[:, cc * T + i * P: cc * T + (i + 1) * P],
                    wq[cc][:, 2 * C:3 * C],
                    start=(cc == 0),
                    stop=(cc == NCT - 1),
                )
            va3 = vaug[i][:, 0:HEADS * VSTRIDE].rearrange(
                "p (h d) -> p h d", d=VSTRIDE)
            nc.vector.tensor_copy(
                va3[:, :, 0:HC],
                ps_v[:].rearrange("p (h d) -> p h d", h=HEADS),
            )

        # prologue part 2: transposes interleaved with pair 0's q/k tiles so
        # the first S^T slot is reachable as early as possible.  Chunk j of
        # qkT[0]/kT2[0] needs x tiles 4j..4j+3 transposed.
        for i in range(NT // 2):
            emit_transpose(i)
        emit_qk(0, 0)
        emit_qk(NCT, 0)
        for i in range(NT // 2, NT):
            emit_transpose(i)
        emit_qk(0, 1)
        emit_qk(NCT, 1)
        tr_cm.__exit__(None, None, None)

        # filler iterator: remaining phase-1 work in dependency-safe order
        def phase1_fillers():
            for m in [1, NCT + 1, 2, NCT + 2, 3, NCT + 3]:
                for j in range(NCH):
                    yield ("qk", m, j)
            for i in range(NT):
                yield ("v", i)

        fillers = phase1_fillers()
        state = {"fill_done": False, "qk_cm": qk_cm, "pv_pool": None}

        def run_fillers(n):
            for _ in range(n):
                try:
                    f = next(fillers)
                except StopIteration:
                    state["fill_done"] = True
                    return
                if f[0] == "qk":
                    emit_qk(f[1], f[2])
                else:
                    emit_v(f[1])

        # ================= phase 2: attention =================
        def emit_pv_chain(h, j, ppv, exh):
            base = (h % 2) * T
            for ssi in range(NT):
                nc.tensor.matmul(
                    ppv[:],
                    vaug[ssi][:, h * VSTRIDE: h * VSTRIDE + P],
                    exh[:, ssi * 2 * T + base + j * CHUNK:
                        ssi * 2 * T + base + (j + 1) * CHUNK],
                    start=(ssi == 0),
                    stop=(ssi == NT - 1),
                )

        def emit_normalize(h, ppv0, ppv1):
            m = h // 2
            rlo = (h % 2) * HC
            dtmp = workp.tile([1, T], F32, tag="dtmp", name="dtmp")
            nc.vector.tensor_copy(dtmp[:, 0:CHUNK], ppv0[HC:HC + 1, :])
            nc.vector.tensor_copy(dtmp[:, CHUNK:T], ppv1[HC:HC + 1, :])
            recip = workp.tile([1, T], F32, tag="recip", name="recip")
            nc.vector.reciprocal_approx_fast(recip[:], dtmp[:])
            bcast = workp.tile([HC, T], F32, tag="bcast", name="bcast")
            nc.gpsimd.partition_broadcast(bcast[:], recip[:], channels=HC)
            for j, ppv in ((0, ppv0), (1, ppv1)):
                nc.vector.tensor_tensor(
                    anorm[m][rlo:rlo + HC, j * CHUNK:(j + 1) * CHUNK],
                    ppv[0:HC, :],
                    bcast[:, j * CHUNK:(j + 1) * CHUNK],
                    op=mybir.AluOpType.mult,
                )

        def emit_pv_half(h, j, ppv, exh, lo):
            base = (h % 2) * T
            for ssi in range(lo, lo + NT // 2):
                nc.tensor.matmul(
                    ppv[:],
                    vaug[ssi][:, h * VSTRIDE: h * VSTRIDE + P],
                    exh[:, ssi * 2 * T + base + j * CHUNK:
                        ssi * 2 * T + base + (j + 1) * CHUNK],
                    start=(ssi == 0),
                    stop=(ssi == NT - 1),
                )

        def make_pv_steps(p, exh):
            """Ten ~0.9us payload units per pair: per head, each PV chunk
            chain is split into two 4-matmul halves, plus one normalize."""
            steps = []
            for hh in range(2):
                h = 2 * p + hh
                box = {}

                def s_alloc0(h=h, box=box):
                    if state["pv_pool"] is None:
                        # opened lazily AFTER the qk pool closes (PSUM budget)
                        state["pv_cm"] = tc.tile_pool(
                            name="ps_pv", bufs=1, space="PSUM", side="right")
                        state["pv_pool"] = state["pv_cm"].__enter__()
                    box["p0"] = state["pv_pool"].tile(
                        [P, CHUNK], F32, tag="pv0", name="ppv0")
                    emit_pv_half(h, 0, box["p0"], exh, 0)

                def s_j0b(h=h, box=box):
                    emit_pv_half(h, 0, box["p0"], exh, NT // 2)

                def s_j1a(h=h, box=box):
                    box["p1"] = state["pv_pool"].tile(
                        [P, CHUNK], F32, tag="pv1", name="ppv1")
                    emit_pv_half(h, 1, box["p1"], exh, 0)

                def s_j1b(h=h, box=box):
                    emit_pv_half(h, 1, box["p1"], exh, NT // 2)

                def s_norm(h=h, box=box):
                    emit_normalize(h, box["p0"], box["p1"])

                steps += [s_alloc0, s_j0b, s_j1a, s_j1b, s_norm]
            return steps

        exh_pool = ctx.enter_context(tc.tile_pool(name="exh", bufs=3))

        # S^T PSUM is a 6-bank ring of 512-wide slices, organized as two
        # 3-slice halves.  exp groups of 3 slices (N=1536) alternate halves,
        # cutting the per-ACTIVATE fixed cost by ~1/3 vs N=1024 calls.
        # Logical slice L (within a pair) = ssi*4 + head*2 + chunk, matching
        # the exh layout, so each group's exp reads/writes contiguous spans.
        RING = 6
        GPP = 11  # groups per pair: 10 x 3 slices + 1 x 2 slices
        # right stack: opened after ps_tr closed, outlives the (left) qk pool
        st_cm = tc.tile_pool(name="ps_st", bufs=1, space="PSUM", side="right")
        st_pool = st_cm.__enter__()
        st_ring = st_pool.tile([P, RING * CHUNK], F32, tag="ring", name="ring")

        exhs = []

        def ensure_exh(p):
            while len(exhs) <= p:
                exhs.append(exh_pool.tile(
                    [P, NT * 2 * T], BF16, tag="exh", name="exh"))

        def group_mms(p, gi):
            """Emit the S^T matmuls for group gi of pair p.  Emission order
            alternates row-group halves so adjacent matmuls run concurrently.
            exp(g) is emitted one group behind, which guarantees a group's
            ring half is never rewritten (g+2) before its exp is ordered."""
            ensure_exh(p)
            Ls = list(range(3 * gi, min(3 * gi + 3, 4 * NT)))
            half = (p * GPP + gi) % 2
            Ls.sort(key=lambda L: (L % 2, (L % 4) // 2))  # j-major, a before b
            for L in Ls:
                ssi, hh, j = L // 4, (L % 4) // 2, L % 2
                pos = L - 3 * gi
                rs = half * 3 + pos
                nc.tensor.matmul(
                    st_ring[:, rs * CHUNK:(rs + 1) * CHUNK],
                    kT2[p][hh * HC:(hh + 1) * HC, ssi * P:(ssi + 1) * P],
                    qkT[p][hh * HC:(hh + 1) * HC, j * CHUNK:(j + 1) * CHUNK],
                    start=True, stop=True,
                )

        def emit_exp_group(p, gi):
            gsz = 3 if gi < GPP - 1 else 2
            half = (p * GPP + gi) % 2
            base = half * 3 * CHUNK
            off = 3 * gi * CHUNK
            nc.scalar.activation(
                exhs[p][:, off: off + gsz * CHUNK],
                st_ring[:, base: base + gsz * CHUNK],
                mybir.ActivationFunctionType.Exp,
                scale=EXP_SCALE,
            )

        pending = []  # queue of PV/normalize closures for the previous pair
        for p in range(NPAIR):
            ensure_exh(p)
            if p > 0:
                pending.extend(make_pv_steps(p - 1, exhs[p - 1]))
                if debug_dumps and p == 1:
                    nc.sync.dma_start(
                        dbg["dbg_exh0"].ap(), exhs[0][:, 0:2 * T])
            for gi in range(GPP):
                group_mms(p, gi)
                # trailing exp: one group behind the matmul stream
                if gi > 0:
                    emit_exp_group(p, gi - 1)
                elif p > 0:
                    emit_exp_group(p - 1, GPP - 1)
                # PE-gap payloads: ~0.9us (one unit) per ~1.5us exp group;
                # fillers run at 2/iter while they last (pair 0 has no PV)
                if not state["fill_done"]:
                    run_fillers(1 + (gi % 2))
                if state["fill_done"]:
                    if state["qk_cm"] is not None:
                        state["qk_cm"].__exit__(None, None, None)
                        state["qk_cm"] = None
                    if pending:
                        pending.pop(0)()
        emit_exp_group(NPAIR - 1, GPP - 1)
        if state["qk_cm"] is not None:
            state["qk_cm"].__exit__(None, None, None)
            state["qk_cm"] = None
        # drain pair 2's PV, release phase-2 PSUM pools
        while pending:
            pending.pop(0)()
        if state["pv_pool"] is not None:
            state["pv_cm"].__exit__(None, None, None)
            state["pv_pool"] = None
        st_cm.__exit__(None, None, None)

        # ---- fast tail: pair 3's PV as bank-parallel chains; head 6's
        # normalize hides under head 7's chains, and head 7's normalize
        # hides under the out-projection (whose dependence on it is isolated
        # to a final K=64 row-tiled accumulation step). ----
        tail_cm = tc.tile_pool(name="ps_tail", bufs=1, space="PSUM")
        tail = tail_cm.__enter__()
        hlast = 2 * (NPAIR - 1)
        exh3 = exhs[-1]
        tp = [tail.tile([P, CHUNK], F32, tag=f"tv{k}", name=f"tv{k}")
              for k in range(4)]

        def tail_chains(hh):
            h = hlast + hh
            for ssi in range(NT):
                for k, j in ((2 * hh, 0), (2 * hh + 1, 1)):
                    nc.tensor.matmul(
                        tp[k][:],
                        vaug[ssi][:, h * VSTRIDE: h * VSTRIDE + P],
                        exh3[:, ssi * 2 * T + hh * T + j * CHUNK:
                             ssi * 2 * T + hh * T + (j + 1) * CHUNK],
                        start=(ssi == 0),
                        stop=(ssi == NT - 1),
                    )

        tail_chains(0)
        tail_chains(1)          # runs while head 6's normalize is on DVE
        emit_normalize(hlast, tp[0], tp[1])
        emit_normalize(hlast + 1, tp[2], tp[3])

        if debug_dumps:
            nc.sync.dma_start(dbg["dbg_xT"].ap(), xT_all[:])
            nc.sync.dma_start(dbg["dbg_q0"].ap(), qkT[0][:])
            nc.sync.dma_start(dbg["dbg_k0"].ap(), kT2[0][:])
            nc.sync.dma_start(dbg["dbg_va0"].ap(), vaug[0][:])
            nc.sync.dma_start(dbg["dbg_an0"].ap(), anorm[0][:])

        # ================= phase 3: out projection =================
        # anorm[3] (pair 3) is the LAST chain input, so cc 0..2 matmuls can
        # run while the tail normalize finishes.
        with tc.tile_pool(name="ps_o", bufs=4, space="PSUM") as ps_op:
            for i in range(NT):
                ps_o = ps_op.tile([P, C], F32, tag="o", name="ps_o")
                for cc in range(NCT):
                    nc.tensor.matmul(
                        ps_o[:],
                        anorm[cc][:, i * P:(i + 1) * P],
                        wo[cc][:],
                        start=(cc == 0),
                        stop=(cc == NCT - 1),
                    )
                osb = workp.tile([P, C], BF16, tag=f"osb{i}", name=f"osb{i}")
                nc.vector.tensor_tensor(
                    osb[:], ps_o[:], ob_bcast[:], op=mybir.AluOpType.add)
                nc.sync.dma_start(out_d.ap()[i * P:(i + 1) * P, :], osb[:])
        tail_cm.__exit__(None, None, None)

    nc.compile()
    return nc


_CACHED_NC = None


def _get_nc():
    global _CACHED_NC
    if _CACHED_NC is None:
        _CACHED_NC = build_program(
            debug_dumps=bool(int(os.environ.get("KERNEL_DEBUG", "0"))))
    return _CACHED_NC


def _prep_inputs(x, qkv_w, qkv_b, out_w, out_b):
    import ml_dtypes

    x = np.asarray(x)
    B = x.shape[0]
    x2 = x.reshape(B, T, C).astype(ml_dtypes.bfloat16)
    wq2 = np.asarray(qkv_w).reshape(C, 3 * C).astype(ml_dtypes.bfloat16)
    wo2 = np.asarray(out_w).reshape(C, C).astype(ml_dtypes.bfloat16)
    qkv_b = np.asarray(qkv_b).astype(np.float32)
    out_b = np.asarray(out_b).astype(np.float32)
    # fold the v-bias through the output projection (exact: A_norm += b_v
    # shifts out by b_v @ W_out since softmax rows sum to 1).
    b_v = qkv_b[2 * C:3 * C]
    ob_eff = (
        out_b.astype(np.float64)
        + b_v.astype(np.float64) @ wo2.astype(np.float64)
    ).astype(np.float32)
    qkb = np.ascontiguousarray(qkv_b[0:2 * C])
    return x2, wq2, wo2, qkb, ob_eff


def kernel(x, qkv_w, qkv_b, out_w, out_b):
    """Full inputs in, full output out.  Shards batch across 8 NeuronCores."""
    from concourse.bass_utils import run_bass_kernel_spmd

    x = np.asarray(x)
    B, H, W, Cc = x.shape
    assert (B, H, W, Cc) == (8, 32, 32, C)
    x2, wq2, wo2, qkb, ob_eff = _prep_inputs(x, qkv_w, qkv_b, out_w, out_b)

    nc = _get_nc()
    in_maps = [
        {
            "x": np.ascontiguousarray(x2[b]),
            "qkv_w": np.ascontiguousarray(wq2),
            "out_w": np.ascontiguousarray(wo2),
            "qk_b": qkb,
            "out_b": ob_eff,
        }
        for b in range(B)
    ]
    trace = bool(int(os.environ.get("KERNEL_TRACE", "0")))
    res = run_bass_kernel_spmd(nc, in_maps, core_ids=list(range(B)), trace=trace)
    if trace and res.exec_time_ns is not None:
        print(f"HW exec time: {res.exec_time_ns} ns")
    kernel.last_results = res
    out = np.stack(
        [np.asarray(res.results[b]["out"]).astype(np.float32) for b in range(B)],
        axis=0,
    )
    return out.reshape(B, H, W, Cc)


kernel.last_results = None


# revision 44
# speedup vs baseline: 1.1448x; 1.1448x over previous
"""Trainium2 Bass kernel for nn_AttentionBlock (B=8, H=W=32, C=512, 8 heads).

Strategy: data-parallel over batch -- each of the 8 NeuronCores processes one
batch element end-to-end (no collectives).  Per core:

  x [T=1024, C=512] -> qkv -> per-head attention (T x T softmax) -> out proj.

v2 design (ACT-bound pipeline):
  * all matmul data is bf16 (host pre-casts); PSUM accumulation stays fp32.
  * S^T = k^T q is computed with K=64 ROW-TILED matmul pairs: head 2p on PE
    row-groups 0-1 (SBUF partitions 0:64), head 2p+1 on row-groups 2-3
    (partitions 64:128).  The two matmuls run concurrently in the array, so
    a head-pair s-tile costs ~2x512 columns instead of 4x512.
  * softmax exp runs on ScalarE (the only exp engine) in N=1024 calls and is
    the phase-2 bottleneck (~73us); everything else (QKV projection, PV,
    out-projection) is woven into the PE gaps between exp calls.
  * denominators come free from a ones-column appended to V (PV row 64);
    normalization = DVE multiply by a GPSIMD-broadcast reciprocal.
  * out-projection uses anorm tiles as lhsT so the output lands directly in
    [t, c] layout -- no output transpose.  Output is bf16; host casts back.
  * no max-subtraction: logits are ~N(0,1) by construction (1/8 scale is
    folded into the ScalarE exp activation).
"""

import math
import os
from contextlib import ExitStack

import numpy as np

import concourse.bass as bass
import concourse.mybir as mybir
import concourse.tile as tile
from concourse import bacc

T = 1024          # tokens per batch element (32*32)
C = 512           # channels
HEADS = 8
HC = C // HEADS   # 64
P = 128           # partitions
NT = T // P       # 8 t-tiles (also 8 s-tiles)
NCT = C // P      # 4 c-tiles
CHUNK = 512       # matmul moving-operand chunk (one fp32 PSUM bank)
NCH = T // CHUNK  # 2 chunks
NPAIR = HEADS // 2
F32 = mybir.dt.float32
BF16 = mybir.dt.bfloat16
EXP_SCALE = 1.0 / math.sqrt(HC)  # (1/sqrt(sqrt(hc)))^2 applied to q.k
VSTRIDE = HC + 1  # 65: v columns + ones column per head
VAW = HEADS * VSTRIDE + (P - VSTRIDE)  # PV lhsT 128-wide reads stay in-tile


def build_program(debug_dumps=False):
    nc = bacc.Bacc("TRN2", num_devices=8, debug=False)

    x_d = nc.dram_tensor("x", [T, C], BF16, kind="ExternalInput")
    wqkv_d = nc.dram_tensor("qkv_w", [C, 3 * C], BF16, kind="ExternalInput")
    wout_d = nc.dram_tensor("out_w", [C, C], BF16, kind="ExternalInput")
    qkb_d = nc.dram_tensor("qk_b", [2 * C], F32, kind="ExternalInput")
    ob_d = nc.dram_tensor("out_b", [C], F32, kind="ExternalInput")
    out_d = nc.dram_tensor("out", [T, C], BF16, kind="ExternalOutput")
    dbg = {}
    if debug_dumps:
        for nm, shp in [
            ("dbg_xT", [P, NCT * T]), ("dbg_q0", [P, T]), ("dbg_k0", [P, T]),
            ("dbg_exh0", [P, 2 * T]), ("dbg_va0", [P, VAW]),
            ("dbg_an0", [P, T]),
        ]:
            dbg[nm] = nc.dram_tensor(nm, shp, BF16, kind="ExternalOutput")

    with tile.TileContext(nc) as tc, ExitStack() as ctx:
        from concourse.masks import make_identity

        # ---------------- SBUF pools ----------------
        const = ctx.enter_context(tc.tile_pool(name="const", bufs=1))
        persist = ctx.enter_context(tc.tile_pool(name="persist", bufs=1))
        workp = ctx.enter_context(tc.tile_pool(name="workp", bufs=1))

        # x in FOUR batched DMAs (2 per HWDGE queue) so the first quarter
        # lands ASAP; per-DMA issue cost is ~0.6us.  Layout:
        # x_in[:, i*C + c] = x[i*128 + p, c] (t-tile-major).
        x_in = persist.tile([P, NT * C], BF16, tag="x_in", name="x_in")
        xr = x_d.ap().rearrange("(i p) c -> p i c", p=P)  # [128, 8, 512]
        xv = x_in[:].rearrange("p (i c) -> p i c", i=NT)
        nc.sync.dma_start(xv[:, 0:2, :], xr[:, 0:2, :])
        nc.scalar.dma_start(xv[:, 2:4, :], xr[:, 2:4, :])
        nc.sync.dma_start(xv[:, 4:6, :], xr[:, 4:6, :])
        nc.scalar.dma_start(xv[:, 6:8, :], xr[:, 6:8, :])

        identity = const.tile([P, P], BF16, tag="ident", name="ident")
        make_identity(nc, identity[:])
        warm_rhs = const.tile([P, CHUNK], BF16, tag="warm", name="warm_rhs")
        nc.gpsimd.memset(warm_rhs[:], 0.0)
        # ones1: K=1 all-ones lhsT for the tail's PE-broadcast normalize
        ones1 = const.tile([1, P], F32, tag="ones1", name="ones1")
        nc.gpsimd.memset(ones1[:], 1.0)

        # qkv weights: q/k columns first (they gate the qk projections),
        # v columns on the slower SWDGE queue afterwards
        wq = []  # [c-tile][128, 1536] bf16
        for m in range(NCT):
            t_ = persist.tile([P, 3 * C], BF16, tag=f"wq{m}", name=f"wq{m}")
            eng = nc.sync if m % 2 == 0 else nc.scalar
            eng.dma_start(t_[:, 0:2 * C],
                          wqkv_d.ap()[m * P:(m + 1) * P, 0:2 * C])
            wq.append(t_)
        for m in range(NCT):
            nc.gpsimd.dma_start(wq[m][:, 2 * C:3 * C],
                                wqkv_d.ap()[m * P:(m + 1) * P, 2 * C:3 * C])

        # bias tiles; column m = qk_b[128m:128m+128]
        qkb_all = const.tile([P, 2 * C // P], F32, tag="qkball", name="qkb_all")
        nc.gpsimd.dma_start(
            qkb_all[:], qkb_d.ap().rearrange("(m p) -> p m", p=P)
        )
        qkb_t = [qkb_all[:, m:m + 1] for m in range(2 * C // P)]
        ob_row = const.tile([1, C], F32, tag="obrow", name="ob_row")
        nc.gpsimd.dma_start(ob_row[:], ob_d.ap().rearrange("(o c) -> o c", o=1))
        ob_bcast = const.tile([P, C], F32, tag="obb", name="ob_bcast")
        nc.gpsimd.partition_broadcast(ob_bcast[:], ob_row[:], channels=P)

        # out-proj weights: single batched DMA, needed only in phase 3
        wo_all = persist.tile([P, NCT * C], BF16, tag="wo", name="wo_all")
        nc.gpsimd.dma_start(
            wo_all[:].rearrange("p (m c) -> p m c", m=NCT),
            wout_d.ap().rearrange("(m p) c -> p m c", p=P),
        )
        wo = [wo_all[:, m * C:(m + 1) * C] for m in range(NCT)]

        # persistent activation tiles
        # xT_all[:, cc*T + t] = x^T tile cc: [c-within-tile, t]
        xT_all = persist.tile([P, NCT * T], BF16, tag="xT", name="xT_all")
        qkT = [persist.tile([P, T], BF16, tag=f"qk{m}", name=f"qk{m}")
               for m in range(NCT)]
        # kT2[m]: rows 0:64 = k^T head 2m, rows 64:128 = k^T head 2m+1
        kT2 = [persist.tile([P, T], BF16, tag=f"k2{m}", name=f"k2{m}")
               for m in range(NCT)]
        vaug = [persist.tile([P, VAW], BF16, tag=f"va{i}", name=f"va{i}")
                for i in range(NT)]
        for i in range(NT):
            nc.gpsimd.memset(vaug[i][:], 1.0)  # ones column (+padding) preset
        anorm = [persist.tile([P, T], BF16, tag=f"an{m}", name=f"an{m}")
                 for m in range(NCT)]

        # ================= prologue =================
        # HAM warm-up: real (non-transpose) matmuls on junk data while the x
        # DMA lands, so the PE clock is at 8/8 when the real work starts.
        with tc.tile_pool(name="ps_warm", bufs=1, space="PSUM") as ps_warm:
            ps_w = ps_warm.tile([P, CHUNK], F32, tag="w", name="ps_w")
            for _ in range(5):
                nc.tensor.matmul(ps_w[:], identity[:], warm_rhs[:],
                                 start=True, stop=True)

        # exp ACT-table preload: a tiny dummy exp during the DMA wait pays the
        # ~2.7us one-time table-load cost before the real pipeline needs it.
        scratch16 = workp.tile([1, 16], F32, tag="scr16", name="scratch16")
        nc.scalar.activation(
            scratch16[:], warm_rhs[0:1, 0:16],
            mybir.ActivationFunctionType.Exp, scale=1.0)

        qk_cm = tc.tile_pool(name="ps_qk", bufs=2, space="PSUM")
        qk_pool = qk_cm.__enter__()

        tr_cm = tc.tile_pool(name="ps_tr", bufs=2, space="PSUM", side="right")
        tr_pool = tr_cm.__enter__()

        def emit_transpose(i):
            # x PE transpose; xT_all[:, cc*T + i*128 : ...] gets tile (i, cc)
            ps_tr = tr_pool.tile([P, C], BF16, tag="tr", name="ps_tr")
            for cc in range(NCT):
                nc.tensor.transpose(
                    ps_tr[:, cc * P:(cc + 1) * P],
                    x_in[:, i * C + cc * P: i * C + (cc + 1) * P],
                    identity[:],
                )
            # one strided copy: dest [128, cc, 128] with cc-stride T
            dst = xT_all[:].rearrange("p (cc t) -> p cc t", cc=NCT)
            nc.vector.tensor_copy(
                dst[:, :, i * P:(i + 1) * P],
                ps_tr[:].rearrange("p (cc q) -> p cc q", cc=NCT),
            )

        def emit_qk(m, j):
            ps_qk = qk_pool.tile([P, CHUNK], F32, tag="qk", name="ps_qk")
            js = slice(j * CHUNK, (j + 1) * CHUNK)
            for cc in range(NCT):
                nc.tensor.matmul(
                    ps_qk[:],
                    wq[cc][:, m * P:(m + 1) * P],
                    xT_all[:, cc * T + j * CHUNK: cc * T + (j + 1) * CHUNK],
                    start=(cc == 0),
                    stop=(cc == NCT - 1),
                )
            dstt = qkT[m] if m < NCT else kT2[m - NCT]
            nc.vector.tensor_scalar_add(dstt[:, js], ps_qk[:], qkb_t[m][:])

        def emit_v(i):
            ps_v = qk_pool.tile([P, CHUNK], F32, tag="qk", name="ps_v")
            for cc in range(NCT):
                nc.tensor.matmul(
                    ps_v[:],
                    xT_all[:, cc * T + i * P: cc * T + (i + 1) * P],
                    wq[cc][:, 2 * C:3 * C],
                    start=(cc == 0),
                    stop=(cc == NCT - 1),
                )
            va3 = vaug[i][:, 0:HEADS * VSTRIDE].rearrange(
                "p (h d) -> p h d", d=VSTRIDE)
            nc.vector.tensor_copy(
                va3[:, :, 0:HC],
                ps_v[:].rearrange("p (h d) -> p h d", h=HEADS),
            )

        # prologue part 2: transposes interleaved with pair 0's q/k tiles so
        # the first S^T slot is reachable as early as possible.  Chunk j of
        # qkT[0]/kT2[0] needs x tiles 4j..4j+3 transposed.
        for i in range(NT // 2):
            emit_transpose(i)
        emit_qk(0, 0)
        emit_qk(NCT, 0)
        for i in range(NT // 2, NT):
            emit_transpose(i)
        emit_qk(0, 1)
        emit_qk(NCT, 1)
        tr_cm.__exit__(None, None, None)

        # filler iterator: remaining phase-1 work in dependency-safe order
        def phase1_fillers():
            for m in [1, NCT + 1, 2, NCT + 2, 3, NCT + 3]:
                for j in range(NCH):
                    yield ("qk", m, j)
            for i in range(NT):
                yield ("v", i)

        fillers = phase1_fillers()
        state = {"fill_done": False, "qk_cm": qk_cm, "pv_pool": None}

        def run_fillers(n):
            for _ in range(n):
                try:
                    f = next(fillers)
                except StopIteration:
                    state["fill_done"] = True
                    return
                if f[0] == "qk":
                    emit_qk(f[1], f[2])
                else:
                    emit_v(f[1])

        # ================= phase 2: attention =================
        def emit_pv_chain(h, j, ppv, exh):
            base = (h % 2) * T
            for ssi in range(NT):
                nc.tensor.matmul(
                    ppv[:],
                    vaug[ssi][:, h * VSTRIDE: h * VSTRIDE + P],
                    exh[:, ssi * 2 * T + base + j * CHUNK:
                        ssi * 2 * T + base + (j + 1) * CHUNK],
                    start=(ssi == 0),
                    stop=(ssi == NT - 1),
                )

        def emit_normalize(h, ppv0, ppv1):
            m = h // 2
            rlo = (h % 2) * HC
            dtmp = workp.tile([1, T], F32, tag="dtmp", name="dtmp")
            nc.vector.tensor_copy(dtmp[:, 0:CHUNK], ppv0[HC:HC + 1, :])
            nc.vector.tensor_copy(dtmp[:, CHUNK:T], ppv1[HC:HC + 1, :])
            recip = workp.tile([1, T], F32, tag="recip", name="recip")
            nc.vector.reciprocal_approx_fast(recip[:], dtmp[:])
            bcast = workp.tile([HC, T], F32, tag="bcast", name="bcast")
            nc.gpsimd.partition_broadcast(bcast[:], recip[:], channels=HC)
            for j, ppv in ((0, ppv0), (1, ppv1)):
                nc.vector.tensor_tensor(
                    anorm[m][rlo:rlo + HC, j * CHUNK:(j + 1) * CHUNK],
                    ppv[0:HC, :],
                    bcast[:, j * CHUNK:(j + 1) * CHUNK],
                    op=mybir.AluOpType.mult,
                )

        def emit_pv_half(h, j, ppv, exh, lo):
            base = (h % 2) * T
            for ssi in range(lo, lo + NT // 2):
                nc.tensor.matmul(
                    ppv[:],
                    vaug[ssi][:, h * VSTRIDE: h * VSTRIDE + P],
                    exh[:, ssi * 2 * T + base + j * CHUNK:
                        ssi * 2 * T + base + (j + 1) * CHUNK],
                    start=(ssi == 0),
                    stop=(ssi == NT - 1),
                )

        def make_pv_steps(p, exh):
            """Ten ~0.9us payload units per pair: per head, each PV chunk
            chain is split into two 4-matmul halves, plus one normalize."""
            steps = []
            for hh in range(2):
                h = 2 * p + hh
                box = {}

                def s_alloc0(h=h, box=box):
                    if state["pv_pool"] is None:
                        # opened lazily AFTER the qk pool closes (PSUM budget)
                        state["pv_cm"] = tc.tile_pool(
                            name="ps_pv", bufs=1, space="PSUM", side="right")
                        state["pv_pool"] = state["pv_cm"].__enter__()
                    box["p0"] = state["pv_pool"].tile(
                        [P, CHUNK], F32, tag="pv0", name="ppv0")
                    emit_pv_half(h, 0, box["p0"], exh, 0)

                def s_j0b(h=h, box=box):
                    emit_pv_half(h, 0, box["p0"], exh, NT // 2)

                def s_j1a(h=h, box=box):
                    box["p1"] = state["pv_pool"].tile(
                        [P, CHUNK], F32, tag="pv1", name="ppv1")
                    emit_pv_half(h, 1, box["p1"], exh, 0)

                def s_j1b(h=h, box=box):
                    emit_pv_half(h, 1, box["p1"], exh, NT // 2)

                def s_norm(h=h, box=box):
                    emit_normalize(h, box["p0"], box["p1"])

                steps += [s_alloc0, s_j0b, s_j1a, s_j1b, s_norm]
            return steps

        exh_pool = ctx.enter_context(tc.tile_pool(name="exh", bufs=3))

        # right stack: opened after ps_tr closed, outlives the (left) qk pool
        st_cm = tc.tile_pool(name="ps_st", bufs=3, space="PSUM", side="right")
        st_pool = st_cm.__enter__()

        slots = [(p, ssi) for p in range(NPAIR) for ssi in range(NT)]
        exhs = []
        st_q = []

        def ensure_exh(p):
            while len(exhs) <= p:
                exhs.append(exh_pool.tile(
                    [P, NT * 2 * T], BF16, tag="exh", name="exh"))

        def emit_st(p, ssi):
            ensure_exh(p)
            sta = st_pool.tile([P, T], F32, tag="st", name="sta")
            stb = st_pool.tile([P, T], F32, tag="st", name="stb")
            for j in range(NCH):
                js = slice(j * CHUNK, (j + 1) * CHUNK)
                nc.tensor.matmul(
                    sta[:, js],
                    kT2[p][0:HC, ssi * P:(ssi + 1) * P],
                    qkT[p][0:HC, js],
                    start=True, stop=True,
                )
                nc.tensor.matmul(
                    stb[:, js],
                    kT2[p][HC:P, ssi * P:(ssi + 1) * P],
                    qkT[p][HC:P, js],
                    start=True, stop=True,
                )
            st_q.append((sta, stb))

        pending = []  # queue of PV/normalize closures for the previous pair
        emit_st(*slots[0])
        for g, (p, ssi) in enumerate(slots):
            exh = exhs[p]
            sta, stb = st_q.pop(0)
            nc.scalar.activation(
                exh[:, ssi * 2 * T: ssi * 2 * T + T],
                sta[:],
                mybir.ActivationFunctionType.Exp,
                scale=EXP_SCALE,
            )
            nc.scalar.activation(
                exh[:, ssi * 2 * T + T: (ssi + 1) * 2 * T],
                stb[:],
                mybir.ActivationFunctionType.Exp,
                scale=EXP_SCALE,
            )
            # next slot's S^T goes in front of this slot's payload work
            if g + 1 < len(slots):
                emit_st(*slots[g + 1])
            if ssi == 0 and p > 0:
                pending.extend(make_pv_steps(p - 1, exhs[p - 1]))
                if debug_dumps and p == 1:
                    nc.sync.dma_start(
                        dbg["dbg_exh0"].ap(), exhs[0][:, 0:2 * T])
            # PE-gap payloads for this slot
            if not state["fill_done"]:
                run_fillers(2)
            if state["fill_done"]:
                if state["qk_cm"] is not None:
                    state["qk_cm"].__exit__(None, None, None)
                    state["qk_cm"] = None
                npop = 2 if len(pending) >= 8 else 1
                for _ in range(npop):
                    if pending:
                        pending.pop(0)()
        if state["qk_cm"] is not None:
            state["qk_cm"].__exit__(None, None, None)
            state["qk_cm"] = None
        # drain pair 2's PV, release phase-2 PSUM pools
        while pending:
            pending.pop(0)()
        if state["pv_pool"] is not None:
            state["pv_cm"].__exit__(None, None, None)
            state["pv_pool"] = None
        st_cm.__exit__(None, None, None)

        # ---- fast tail: pair 3's PV as bank-parallel chains; head 6's
        # normalize hides under head 7's chains, and head 7's normalize
        # hides under the out-projection (whose dependence on it is isolated
        # to a final K=64 row-tiled accumulation step). ----
        tail_cm = tc.tile_pool(name="ps_tail", bufs=1, space="PSUM")
        tail = tail_cm.__enter__()
        hlast = 2 * (NPAIR - 1)
        exh3 = exhs[-1]
        tp = [tail.tile([P, CHUNK], F32, tag=f"tv{k}", name=f"tv{k}")
              for k in range(4)]

        def tail_chains(hh):
            h = hlast + hh
            for ssi in range(NT):
                for k, j in ((2 * hh, 0), (2 * hh + 1, 1)):
                    nc.tensor.matmul(
                        tp[k][:],
                        vaug[ssi][:, h * VSTRIDE: h * VSTRIDE + P],
                        exh3[:, ssi * 2 * T + hh * T + j * CHUNK:
                             ssi * 2 * T + hh * T + (j + 1) * CHUNK],
                        start=(ssi == 0),
                        stop=(ssi == NT - 1),
                    )

        tail_chains(0)
        tail_chains(1)          # runs while head 6's normalize is on DVE
        emit_normalize(hlast, tp[0], tp[1])
        emit_normalize(hlast + 1, tp[2], tp[3])

        if debug_dumps:
            nc.sync.dma_start(dbg["dbg_xT"].ap(), xT_all[:])
            nc.sync.dma_start(dbg["dbg_q0"].ap(), qkT[0][:])
            nc.sync.dma_start(dbg["dbg_k0"].ap(), kT2[0][:])
            nc.sync.dma_start(dbg["dbg_va0"].ap(), vaug[0][:])
            nc.sync.dma_start(dbg["dbg_an0"].ap(), anorm[0][:])

        # ================= phase 3: out projection =================
        # anorm[3] (pair 3) is the LAST chain input, so cc 0..2 matmuls can
        # run while the tail normalize finishes.
        with tc.tile_pool(name="ps_o", bufs=4, space="PSUM") as ps_op:
            for i in range(NT):
                ps_o = ps_op.tile([P, C], F32, tag="o", name="ps_o")
                for cc in range(NCT):
                    nc.tensor.matmul(
                        ps_o[:],
                        anorm[cc][:, i * P:(i + 1) * P],
                        wo[cc][:],
                        start=(cc == 0),
                        stop=(cc == NCT - 1),
                    )
                osb = workp.tile([P, C], BF16, tag=f"osb{i}", name=f"osb{i}")
                nc.vector.tensor_tensor(
                    osb[:], ps_o[:], ob_bcast[:], op=mybir.AluOpType.add)
                nc.sync.dma_start(out_d.ap()[i * P:(i + 1) * P, :], osb[:])
        tail_cm.__exit__(None, None, None)

    nc.compile()
    return nc


_CACHED_NC = None


def _get_nc():
    global _CACHED_NC
    if _CACHED_NC is None:
        _CACHED_NC = build_program(
            debug_dumps=bool(int(os.environ.get("KERNEL_DEBUG", "0"))))
    return _CACHED_NC


def _prep_inputs(x, qkv_w, qkv_b, out_w, out_b):
    import ml_dtypes

    x = np.asarray(x)
    B = x.shape[0]
    x2 = x.reshape(B, T, C).astype(ml_dtypes.bfloat16)
    wq2 = np.asarray(qkv_w).reshape(C, 3 * C).astype(ml_dtypes.bfloat16)
    wo2 = np.asarray(out_w).reshape(C, C).astype(ml_dtypes.bfloat16)
    qkv_b = np.asarray(qkv_b).astype(np.float32)
    out_b = np.asarray(out_b).astype(np.float32)
    # fold the v-bias through the output projection (exact: A_norm += b_v
    # shifts out by b_v @ W_out since softmax rows sum to 1).
    b_v = qkv_b[2 * C:3 * C]
    ob_eff = (
        out_b.astype(np.float64)
        + b_v.astype(np.float64) @ wo2.astype(np.float64)
    ).astype(np.float32)
    qkb = np.ascontiguousarray(qkv_b[0:2 * C])
    return x2, wq2, wo2, qkb, ob_eff


def kernel(x, qkv_w, qkv_b, out_w, out_b):
    """Full inputs in, full output out.  Shards batch across 8 NeuronCores."""
    from concourse.bass_utils import run_bass_kernel_spmd

    x = np.asarray(x)
    B, H, W, Cc = x.shape
    assert (B, H, W, Cc) == (8, 32, 32, C)
    x2, wq2, wo2, qkb, ob_eff = _prep_inputs(x, qkv_w, qkv_b, out_w, out_b)

    nc = _get_nc()
    in_maps = [
        {
            "x": np.ascontiguousarray(x2[b]),
            "qkv_w": np.ascontiguousarray(wq2),
            "out_w": np.ascontiguousarray(wo2),
            "qk_b": qkb,
            "out_b": ob_eff,
        }
        for b in range(B)
    ]
    trace = bool(int(os.environ.get("KERNEL_TRACE", "0")))
    res = run_bass_kernel_spmd(nc, in_maps, core_ids=list(range(B)), trace=trace)
    if trace and res.exec_time_ns is not None:
        print(f"HW exec time: {res.exec_time_ns} ns")
    kernel.last_results = res
    out = np.stack(
        [np.asarray(res.results[b]["out"]).astype(np.float32) for b in range(B)],
        axis=0,
    )
    return out.reshape(B, H, W, Cc)


kernel.last_results = None


# revision 45
# speedup vs baseline: 1.1739x; 1.0254x over previous
"""Trainium2 Bass kernel for nn_AttentionBlock (B=8, H=W=32, C=512, 8 heads).

Strategy: data-parallel over batch -- each of the 8 NeuronCores processes one
batch element end-to-end (no collectives).  Per core:

  x [T=1024, C=512] -> qkv -> per-head attention (T x T softmax) -> out proj.

v2 design (ACT-bound pipeline):
  * all matmul data is bf16 (host pre-casts); PSUM accumulation stays fp32.
  * S^T = k^T q is computed with K=64 ROW-TILED matmul pairs: head 2p on PE
    row-groups 0-1 (SBUF partitions 0:64), head 2p+1 on row-groups 2-3
    (partitions 64:128).  The two matmuls run concurrently in the array, so
    a head-pair s-tile costs ~2x512 columns instead of 4x512.
  * softmax exp runs on ScalarE (the only exp engine) in N=1024 calls and is
    the phase-2 bottleneck (~73us); everything else (QKV projection, PV,
    out-projection) is woven into the PE gaps between exp calls.
  * denominators come free from a ones-column appended to V (PV row 64);
    normalization = DVE multiply by a GPSIMD-broadcast reciprocal.
  * out-projection uses anorm tiles as lhsT so the output lands directly in
    [t, c] layout -- no output transpose.  Output is bf16; host casts back.
  * no max-subtraction: logits are ~N(0,1) by construction (1/8 scale is
    folded into the ScalarE exp activation).
"""

import math
import os
from contextlib import ExitStack

import numpy as np

import concourse.bass as bass
import concourse.mybir as mybir
import concourse.tile as tile
from concourse import bacc

T = 1024          # tokens per batch element (32*32)
C = 512           # channels
HEADS = 8
HC = C // HEADS   # 64
P = 128           # partitions
NT = T // P       # 8 t-tiles (also 8 s-tiles)
NCT = C // P      # 4 c-tiles
CHUNK = 512       # matmul moving-operand chunk (one fp32 PSUM bank)
NCH = T // CHUNK  # 2 chunks
NPAIR = HEADS // 2
F32 = mybir.dt.float32
BF16 = mybir.dt.bfloat16
EXP_SCALE = 1.0 / math.sqrt(HC)  # (1/sqrt(sqrt(hc)))^2 applied to q.k
VSTRIDE = HC + 1  # 65: v columns + ones column per head
VAW = HEADS * VSTRIDE + (P - VSTRIDE)  # PV lhsT 128-wide reads stay in-tile


def build_program(debug_dumps=False):
    nc = bacc.Bacc("TRN2", num_devices=8, debug=False)

    x_d = nc.dram_tensor("x", [T, C], BF16, kind="ExternalInput")
    wqkv_d = nc.dram_tensor("qkv_w", [C, 3 * C], BF16, kind="ExternalInput")
    wout_d = nc.dram_tensor("out_w", [C, C], BF16, kind="ExternalInput")
    qkb_d = nc.dram_tensor("qk_b", [2 * C], F32, kind="ExternalInput")
    ob_d = nc.dram_tensor("out_b", [C], F32, kind="ExternalInput")
    out_d = nc.dram_tensor("out", [T, C], BF16, kind="ExternalOutput")
    dbg = {}
    if debug_dumps:
        for nm, shp in [
            ("dbg_xT", [P, NCT * T]), ("dbg_q0", [P, T]), ("dbg_k0", [P, T]),
            ("dbg_exh0", [P, 2 * T]), ("dbg_va0", [P, VAW]),
            ("dbg_an0", [P, T]),
        ]:
            dbg[nm] = nc.dram_tensor(nm, shp, BF16, kind="ExternalOutput")

    with tile.TileContext(nc) as tc, ExitStack() as ctx:
        from concourse.masks import make_identity

        # ---------------- SBUF pools ----------------
        const = ctx.enter_context(tc.tile_pool(name="const", bufs=1))
        persist = ctx.enter_context(tc.tile_pool(name="persist", bufs=1))
        workp = ctx.enter_context(tc.tile_pool(name="workp", bufs=1))

        # x in FOUR batched DMAs (2 per HWDGE queue) so the first quarter
        # lands ASAP; per-DMA issue cost is ~0.6us.  Layout:
        # x_in[:, i*C + c] = x[i*128 + p, c] (t-tile-major).
        x_in = persist.tile([P, NT * C], BF16, tag="x_in", name="x_in")
        xr = x_d.ap().rearrange("(i p) c -> p i c", p=P)  # [128, 8, 512]
        xv = x_in[:].rearrange("p (i c) -> p i c", i=NT)
        nc.sync.dma_start(xv[:, 0:2, :], xr[:, 0:2, :])
        nc.scalar.dma_start(xv[:, 2:4, :], xr[:, 2:4, :])
        nc.sync.dma_start(xv[:, 4:6, :], xr[:, 4:6, :])
        nc.scalar.dma_start(xv[:, 6:8, :], xr[:, 6:8, :])

        identity = const.tile([P, P], BF16, tag="ident", name="ident")
        make_identity(nc, identity[:])
        warm_rhs = const.tile([P, CHUNK], BF16, tag="warm", name="warm_rhs")
        nc.gpsimd.memset(warm_rhs[:], 0.0)
        # ones1: K=1 all-ones lhsT for the tail's PE-broadcast normalize
        ones1 = const.tile([1, P], F32, tag="ones1", name="ones1")
        nc.gpsimd.memset(ones1[:], 1.0)

        # qkv weights: q/k columns first (they gate the qk projections),
        # v columns on the slower SWDGE queue afterwards
        wq = []  # [c-tile][128, 1536] bf16
        for m in range(NCT):
            t_ = persist.tile([P, 3 * C], BF16, tag=f"wq{m}", name=f"wq{m}")
            eng = nc.sync if m % 2 == 0 else nc.scalar
            eng.dma_start(t_[:, 0:2 * C],
                          wqkv_d.ap()[m * P:(m + 1) * P, 0:2 * C])
            wq.append(t_)
        # v columns + out-proj weights queue BEHIND the critical x/q/k
        # transfers on the same HWDGE rings (each ring drains in FIFO order,
        # so critical data gets the HBM bandwidth first)
        for m in range(NCT):
            eng = nc.sync if m % 2 == 0 else nc.scalar
            eng.dma_start(wq[m][:, 2 * C:3 * C],
                          wqkv_d.ap()[m * P:(m + 1) * P, 2 * C:3 * C])

        # bias tiles; column m = qk_b[128m:128m+128]
        qkb_all = const.tile([P, 2 * C // P], F32, tag="qkball", name="qkb_all")
        nc.gpsimd.dma_start(
            qkb_all[:], qkb_d.ap().rearrange("(m p) -> p m", p=P)
        )
        qkb_t = [qkb_all[:, m:m + 1] for m in range(2 * C // P)]
        ob_row = const.tile([1, C], F32, tag="obrow", name="ob_row")
        nc.gpsimd.dma_start(ob_row[:], ob_d.ap().rearrange("(o c) -> o c", o=1))
        ob_bcast = const.tile([P, C], F32, tag="obb", name="ob_bcast")
        nc.gpsimd.partition_broadcast(ob_bcast[:], ob_row[:], channels=P)

        # out-proj weights: single batched DMA, needed only in phase 3
        wo_all = persist.tile([P, NCT * C], BF16, tag="wo", name="wo_all")
        nc.sync.dma_start(
            wo_all[:].rearrange("p (m c) -> p m c", m=NCT),
            wout_d.ap().rearrange("(m p) c -> p m c", p=P),
        )
        wo = [wo_all[:, m * C:(m + 1) * C] for m in range(NCT)]

        # persistent activation tiles
        # xT_all[:, cc*T + t] = x^T tile cc: [c-within-tile, t]
        xT_all = persist.tile([P, NCT * T], BF16, tag="xT", name="xT_all")
        qkT = [persist.tile([P, T], BF16, tag=f"qk{m}", name=f"qk{m}")
               for m in range(NCT)]
        # kT2[m]: rows 0:64 = k^T head 2m, rows 64:128 = k^T head 2m+1
        kT2 = [persist.tile([P, T], BF16, tag=f"k2{m}", name=f"k2{m}")
               for m in range(NCT)]
        vaug = [persist.tile([P, VAW], BF16, tag=f"va{i}", name=f"va{i}")
                for i in range(NT)]
        for i in range(NT):
            nc.gpsimd.memset(vaug[i][:], 1.0)  # ones column (+padding) preset
        anorm = [persist.tile([P, T], BF16, tag=f"an{m}", name=f"an{m}")
                 for m in range(NCT)]

        # ================= prologue =================
        # HAM warm-up: real (non-transpose) matmuls on junk data while the x
        # DMA lands, so the PE clock is at 8/8 when the real work starts.
        with tc.tile_pool(name="ps_warm", bufs=1, space="PSUM") as ps_warm:
            ps_w = ps_warm.tile([P, CHUNK], F32, tag="w", name="ps_w")
            for _ in range(5):
                nc.tensor.matmul(ps_w[:], identity[:], warm_rhs[:],
                                 start=True, stop=True)

        # exp ACT-table preload: a tiny dummy exp during the DMA wait pays the
        # ~2.7us one-time table-load cost before the real pipeline needs it.
        scratch16 = workp.tile([1, 16], F32, tag="scr16", name="scratch16")
        nc.scalar.activation(
            scratch16[:], warm_rhs[0:1, 0:16],
            mybir.ActivationFunctionType.Exp, scale=1.0)

        qk_cm = tc.tile_pool(name="ps_qk", bufs=2, space="PSUM")
        qk_pool = qk_cm.__enter__()

        tr_cm = tc.tile_pool(name="ps_tr", bufs=2, space="PSUM", side="right")
        tr_pool = tr_cm.__enter__()

        def emit_transpose(i):
            # x PE transpose; xT_all[:, cc*T + i*128 : ...] gets tile (i, cc)
            ps_tr = tr_pool.tile([P, C], BF16, tag="tr", name="ps_tr")
            for cc in range(NCT):
                nc.tensor.transpose(
                    ps_tr[:, cc * P:(cc + 1) * P],
                    x_in[:, i * C + cc * P: i * C + (cc + 1) * P],
                    identity[:],
                )
            # one strided copy: dest [128, cc, 128] with cc-stride T
            dst = xT_all[:].rearrange("p (cc t) -> p cc t", cc=NCT)
            nc.vector.tensor_copy(
                dst[:, :, i * P:(i + 1) * P],
                ps_tr[:].rearrange("p (cc q) -> p cc q", cc=NCT),
            )

        def emit_qk(m, j):
            ps_qk = qk_pool.tile([P, CHUNK], F32, tag="qk", name="ps_qk")
            js = slice(j * CHUNK, (j + 1) * CHUNK)
            for cc in range(NCT):
                nc.tensor.matmul(
                    ps_qk[:],
                    wq[cc][:, m * P:(m + 1) * P],
                    xT_all[:, cc * T + j * CHUNK: cc * T + (j + 1) * CHUNK],
                    start=(cc == 0),
                    stop=(cc == NCT - 1),
                )
            dstt = qkT[m] if m < NCT else kT2[m - NCT]
            nc.vector.tensor_scalar_add(dstt[:, js], ps_qk[:], qkb_t[m][:])

        def emit_v(i):
            ps_v = qk_pool.tile([P, CHUNK], F32, tag="qk", name="ps_v")
            for cc in range(NCT):
                nc.tensor.matmul(
                    ps_v[:],
                    xT_all[:, cc * T + i * P: cc * T + (i + 1) * P],
                    wq[cc][:, 2 * C:3 * C],
                    start=(cc == 0),
                    stop=(cc == NCT - 1),
                )
            va3 = vaug[i][:, 0:HEADS * VSTRIDE].rearrange(
                "p (h d) -> p h d", d=VSTRIDE)
            nc.vector.tensor_copy(
                va3[:, :, 0:HC],
                ps_v[:].rearrange("p (h d) -> p h d", h=HEADS),
            )

        # prologue part 2: transposes interleaved with pair 0's q/k tiles so
        # the first S^T slot is reachable as early as possible.  Chunk j of
        # qkT[0]/kT2[0] needs x tiles 4j..4j+3 transposed.
        for i in range(NT // 2):
            emit_transpose(i)
        emit_qk(0, 0)
        emit_qk(NCT, 0)
        for i in range(NT // 2, NT):
            emit_transpose(i)
        emit_qk(0, 1)
        emit_qk(NCT, 1)
        tr_cm.__exit__(None, None, None)
        pv_cm = tc.tile_pool(name="ps_pv", bufs=1, space="PSUM", side="right")
        state_pv_pool = pv_cm.__enter__()

        # filler iterator: remaining phase-1 work in dependency-safe order
        def phase1_fillers():
            for m in [1, NCT + 1, 2, NCT + 2, 3, NCT + 3]:
                for j in range(NCH):
                    yield ("qk", m, j)
            for i in range(NT):
                yield ("v", i)

        fillers = phase1_fillers()
        state = {"fill_done": False, "qk_cm": qk_cm, "pv_pool": None}

        def run_fillers(n):
            for _ in range(n):
                try:
                    f = next(fillers)
                except StopIteration:
                    state["fill_done"] = True
                    return
                if f[0] == "qk":
                    emit_qk(f[1], f[2])
                else:
                    emit_v(f[1])

        # ================= phase 2: attention =================
        def emit_pv_chain(h, j, ppv, exh):
            base = (h % 2) * T
            for ssi in range(NT):
                nc.tensor.matmul(
                    ppv[:],
                    vaug[ssi][:, h * VSTRIDE: h * VSTRIDE + P],
                    exh[:, ssi * 2 * T + base + j * CHUNK:
                        ssi * 2 * T + base + (j + 1) * CHUNK],
                    start=(ssi == 0),
                    stop=(ssi == NT - 1),
                )

        def emit_normalize(h, ppv0, ppv1):
            m = h // 2
            rlo = (h % 2) * HC
            dtmp = workp.tile([1, T], F32, tag="dtmp", name="dtmp")
            nc.vector.tensor_copy(dtmp[:, 0:CHUNK], ppv0[HC:HC + 1, :])
            nc.vector.tensor_copy(dtmp[:, CHUNK:T], ppv1[HC:HC + 1, :])
            recip = workp.tile([1, T], F32, tag="recip", name="recip")
            nc.vector.reciprocal_approx_fast(recip[:], dtmp[:])
            bcast = workp.tile([HC, T], F32, tag="bcast", name="bcast")
            nc.gpsimd.partition_broadcast(bcast[:], recip[:], channels=HC)
            for j, ppv in ((0, ppv0), (1, ppv1)):
                nc.vector.tensor_tensor(
                    anorm[m][rlo:rlo + HC, j * CHUNK:(j + 1) * CHUNK],
                    ppv[0:HC, :],
                    bcast[:, j * CHUNK:(j + 1) * CHUNK],
                    op=mybir.AluOpType.mult,
                )

        def emit_pv_half(h, j, ppv, exh, lo):
            base = (h % 2) * T
            for ssi in range(lo, lo + NT // 2):
                nc.tensor.matmul(
                    ppv[:],
                    vaug[ssi][:, h * VSTRIDE: h * VSTRIDE + P],
                    exh[:, ssi * 2 * T + base + j * CHUNK:
                        ssi * 2 * T + base + (j + 1) * CHUNK],
                    start=(ssi == 0),
                    stop=(ssi == NT - 1),
                )

        def make_pv_steps(p, exh):
            """Ten ~0.9us payload units per pair: per head, each PV chunk
            chain is split into two 4-matmul halves, plus one normalize."""
            steps = []
            for hh in range(2):
                h = 2 * p + hh
                box = {}

                def s_alloc0(h=h, box=box):
                    box["p0"] = state_pv_pool.tile(
                        [P, CHUNK], F32, tag="pv0", name="ppv0")
                    emit_pv_half(h, 0, box["p0"], exh, 0)

                def s_j0b(h=h, box=box):
                    emit_pv_half(h, 0, box["p0"], exh, NT // 2)

                def s_j1a(h=h, box=box):
                    box["p1"] = state_pv_pool.tile(
                        [P, CHUNK], F32, tag="pv1", name="ppv1")
                    emit_pv_half(h, 1, box["p1"], exh, 0)

                def s_j1b(h=h, box=box):
                    emit_pv_half(h, 1, box["p1"], exh, NT // 2)

                def s_norm(h=h, box=box):
                    emit_normalize(h, box["p0"], box["p1"])

                steps += [s_alloc0, s_j0b, s_j1a, s_j1b, s_norm]
            return steps

        exh_pool = ctx.enter_context(tc.tile_pool(name="exh", bufs=3))

        # right stack: opened after ps_tr closed, outlives the (left) qk pool
        st_cm = tc.tile_pool(name="ps_st", bufs=2, space="PSUM", side="right")
        st_pool = st_cm.__enter__()

        slots = [(p, ssi) for p in range(NPAIR) for ssi in range(NT)]
        exhs = []
        st_q = []

        def ensure_exh(p):
            while len(exhs) <= p:
                exhs.append(exh_pool.tile(
                    [P, NT * 2 * T], BF16, tag="exh", name="exh"))

        def emit_st(p, ssi):
            ensure_exh(p)
            sta = st_pool.tile([P, T], F32, tag="st", name="sta")
            stb = st_pool.tile([P, T], F32, tag="st", name="stb")
            for j in range(NCH):
                js = slice(j * CHUNK, (j + 1) * CHUNK)
                nc.tensor.matmul(
                    sta[:, js],
                    kT2[p][0:HC, ssi * P:(ssi + 1) * P],
                    qkT[p][0:HC, js],
                    start=True, stop=True,
                )
                nc.tensor.matmul(
                    stb[:, js],
                    kT2[p][HC:P, ssi * P:(ssi + 1) * P],
                    qkT[p][HC:P, js],
                    start=True, stop=True,
                )
            st_q.append((sta, stb))

        pending = []  # queue of PV/normalize closures for the previous pair
        emit_st(*slots[0])
        for g, (p, ssi) in enumerate(slots):
            exh = exhs[p]
            sta, stb = st_q.pop(0)
            nc.scalar.activation(
                exh[:, ssi * 2 * T: ssi * 2 * T + T],
                sta[:],
                mybir.ActivationFunctionType.Exp,
                scale=EXP_SCALE,
            )
            nc.scalar.activation(
                exh[:, ssi * 2 * T + T: (ssi + 1) * 2 * T],
                stb[:],
                mybir.ActivationFunctionType.Exp,
                scale=EXP_SCALE,
            )
            # next slot's S^T goes in front of this slot's payload work
            if g + 1 < len(slots):
                emit_st(*slots[g + 1])
            if ssi == 0 and p > 0:
                pending.extend(make_pv_steps(p - 1, exhs[p - 1]))
                if debug_dumps and p == 1:
                    nc.sync.dma_start(
                        dbg["dbg_exh0"].ap(), exhs[0][:, 0:2 * T])
            if ssi == 4 and p == NPAIR - 1:
                # pair 3's own PV joins the payload queue (its early-s-tile
                # halves only need already-finished exps); keeps the PE warm
                # through the last slots and shrinks the tail
                pending.extend(make_pv_steps(p, exh))
            # PE-gap payloads for this slot
            if not state["fill_done"]:
                run_fillers(2)
            if state["fill_done"]:
                if state["qk_cm"] is not None:
                    state["qk_cm"].__exit__(None, None, None)
                    state["qk_cm"] = None
                npop = 2 if len(pending) >= 8 else 1
                for _ in range(npop):
                    if pending:
                        pending.pop(0)()
        if state["qk_cm"] is not None:
            state["qk_cm"].__exit__(None, None, None)
            state["qk_cm"] = None
        # drain all remaining PV/normalize steps except pair 3's final
        # normalize, which overlaps the out-projection's cc0-2 matmuls
        while len(pending) > 1:
            pending.pop(0)()
        st_cm.__exit__(None, None, None)

        if debug_dumps:
            nc.sync.dma_start(dbg["dbg_xT"].ap(), xT_all[:])
            nc.sync.dma_start(dbg["dbg_q0"].ap(), qkT[0][:])
            nc.sync.dma_start(dbg["dbg_k0"].ap(), kT2[0][:])
            nc.sync.dma_start(dbg["dbg_va0"].ap(), vaug[0][:])

        # ================= phase 3: out projection =================
        ps_o_cm = tc.tile_pool(name="ps_o", bufs=4, space="PSUM")
        ps_op = ps_o_cm.__enter__()
        otiles = []
        for i in range(4):
            ps_o = ps_op.tile([P, C], F32, tag="o", name="ps_o")
            otiles.append(ps_o)
            for cc in range(NCT - 1):
                nc.tensor.matmul(
                    ps_o[:],
                    anorm[cc][:, i * P:(i + 1) * P],
                    wo[cc][:],
                    start=(cc == 0),
                    stop=False,
                )
        if pending:
            pending.pop(0)()  # pair 3's last normalize

        def finish_tile(i, ps_o):
            nc.tensor.matmul(
                ps_o[:],
                anorm[NCT - 1][:, i * P:(i + 1) * P],
                wo[NCT - 1][:],
                start=False,
                stop=True,
            )
            osb = workp.tile([P, C], BF16, tag=f"osb{i}", name=f"osb{i}")
            nc.vector.tensor_tensor(
                osb[:], ps_o[:], ob_bcast[:], op=mybir.AluOpType.add)
            nc.sync.dma_start(out_d.ap()[i * P:(i + 1) * P, :], osb[:])

        for i in range(4):
            finish_tile(i, otiles[i])
        for i in range(4, NT):
            ps_o = ps_op.tile([P, C], F32, tag="o", name="ps_o")
            for cc in range(NCT - 1):
                nc.tensor.matmul(
                    ps_o[:],
                    anorm[cc][:, i * P:(i + 1) * P],
                    wo[cc][:],
                    start=(cc == 0),
                    stop=False,
                )
            finish_tile(i, ps_o)
        ps_o_cm.__exit__(None, None, None)
        pv_cm.__exit__(None, None, None)
        if debug_dumps:
            nc.sync.dma_start(dbg["dbg_an0"].ap(), anorm[0][:])

    nc.compile()
    return nc


_CACHED_NC = None


def _get_nc():
    global _CACHED_NC
    if _CACHED_NC is None:
        _CACHED_NC = build_program(
            debug_dumps=bool(int(os.environ.get("KERNEL_DEBUG", "0"))))
    return _CACHED_NC


def _prep_inputs(x, qkv_w, qkv_b, out_w, out_b):
    import ml_dtypes

    x = np.asarray(x)
    B = x.shape[0]
    x2 = x.reshape(B, T, C).astype(ml_dtypes.bfloat16)
    wq2 = np.asarray(qkv_w).reshape(C, 3 * C).astype(ml_dtypes.bfloat16)
    wo2 = np.asarray(out_w).reshape(C, C).astype(ml_dtypes.bfloat16)
    qkv_b = np.asarray(qkv_b).astype(np.float32)
    out_b = np.asarray(out_b).astype(np.float32)
    # fold the v-bias through the output projection (exact: A_norm += b_v
    # shifts out by b_v @ W_out since softmax rows sum to 1).
    b_v = qkv_b[2 * C:3 * C]
    ob_eff = (
        out_b.astype(np.float64)
        + b_v.astype(np.float64) @ wo2.astype(np.float64)
    ).astype(np.float32)
    qkb = np.ascontiguousarray(qkv_b[0:2 * C])
    return x2, wq2, wo2, qkb, ob_eff


def kernel(x, qkv_w, qkv_b, out_w, out_b):
    """Full inputs in, full output out.  Shards batch across 8 NeuronCores."""
    from concourse.bass_utils import run_bass_kernel_spmd

    x = np.asarray(x)
    B, H, W, Cc = x.shape
    assert (B, H, W, Cc) == (8, 32, 32, C)
    x2, wq2, wo2, qkb, ob_eff = _prep_inputs(x, qkv_w, qkv_b, out_w, out_b)

    nc = _get_nc()
    in_maps = [
        {
            "x": np.ascontiguousarray(x2[b]),
            "qkv_w": np.ascontiguousarray(wq2),
            "out_w": np.ascontiguousarray(wo2),
            "qk_b": qkb,
            "out_b": ob_eff,
        }
        for b in range(B)
    ]
    trace = bool(int(os.environ.get("KERNEL_TRACE", "0")))
    res = run_bass_kernel_spmd(nc, in_maps, core_ids=list(range(B)), trace=trace)
    if trace and res.exec_time_ns is not None:
        print(f"HW exec time: {res.exec_time_ns} ns")
    kernel.last_results = res
    out = np.stack(
        [np.asarray(res.results[b]["out"]).astype(np.float32) for b in range(B)],
        axis=0,
    )
    return out.reshape(B, H, W, Cc)


kernel.last_results = None


# revision 46
# speedup vs baseline: 1.3673x; 1.1648x over previous
"""Trainium2 Bass kernel for nn_AttentionBlock (B=8, H=W=32, C=512, 8 heads).

Strategy: data-parallel over batch -- each of the 8 NeuronCores processes one
batch element end-to-end (no collectives).  Per core:

  x [T=1024, C=512] -> qkv -> per-head attention (T x T softmax) -> out proj.

v2 design (ACT-bound pipeline):
  * all matmul data is bf16 (host pre-casts); PSUM accumulation stays fp32.
  * S^T = k^T q is computed with K=64 ROW-TILED matmul pairs: head 2p on PE
    row-groups 0-1 (SBUF partitions 0:64), head 2p+1 on row-groups 2-3
    (partitions 64:128).  The two matmuls run concurrently in the array, so
    a head-pair s-tile costs ~2x512 columns instead of 4x512.
  * softmax exp runs on ScalarE (the only exp engine) in N=1024 calls and is
    the phase-2 bottleneck (~73us); everything else (QKV projection, PV,
    out-projection) is woven into the PE gaps between exp calls.
  * denominators come free from a ones-column appended to V (PV row 64);
    normalization = DVE multiply by a GPSIMD-broadcast reciprocal.
  * out-projection uses anorm tiles as lhsT so the output lands directly in
    [t, c] layout -- no output transpose.  Output is bf16; host casts back.
  * no max-subtraction: logits are ~N(0,1) by construction (1/8 scale is
    folded into the ScalarE exp activation).
"""

import math
import os
from contextlib import ExitStack

import numpy as np

import concourse.bass as bass
import concourse.mybir as mybir
import concourse.tile as tile
from concourse import bacc

T = 1024          # tokens per batch element (32*32)
C = 512           # channels
HEADS = 8
HC = C // HEADS   # 64
P = 128           # partitions
NT = T // P       # 8 t-tiles (also 8 s-tiles)
NCT = C // P      # 4 c-tiles
CHUNK = 512       # matmul moving-operand chunk (one fp32 PSUM bank)
NCH = T // CHUNK  # 2 chunks
NPAIR = HEADS // 2
F32 = mybir.dt.float32
BF16 = mybir.dt.bfloat16
EXP_SCALE = 1.0 / math.sqrt(HC)  # (1/sqrt(sqrt(hc)))^2 applied to q.k
VSTRIDE = HC + 1  # 65: v columns + ones column per head
VAW = HEADS * VSTRIDE + (P - VSTRIDE)  # PV lhsT 128-wide reads stay in-tile


def build_program(debug_dumps=False):
    nc = bacc.Bacc("TRN2", num_devices=8, debug=False)

    x_d = nc.dram_tensor("x", [T, C], BF16, kind="ExternalInput")
    wqkv_d = nc.dram_tensor("qkv_w", [C, 3 * C], BF16, kind="ExternalInput")
    wout_d = nc.dram_tensor("out_w", [C, C], BF16, kind="ExternalInput")
    qkb_d = nc.dram_tensor("qk_b", [2 * C], F32, kind="ExternalInput")
    ob_d = nc.dram_tensor("out_b", [C], F32, kind="ExternalInput")
    out_d = nc.dram_tensor("out", [T, C], BF16, kind="ExternalOutput")
    dbg = {}
    if debug_dumps:
        for nm, shp in [
            ("dbg_xT", [P, NCT * T]), ("dbg_q0", [P, T]), ("dbg_k0", [P, T]),
            ("dbg_exh0", [P, 2 * T]), ("dbg_va0", [P, VAW]),
            ("dbg_an0", [P, T]),
        ]:
            dbg[nm] = nc.dram_tensor(nm, shp, BF16, kind="ExternalOutput")

    with tile.TileContext(nc) as tc, ExitStack() as ctx:
        from concourse.masks import make_identity

        # ---------------- SBUF pools ----------------
        const = ctx.enter_context(tc.tile_pool(name="const", bufs=1))
        persist = ctx.enter_context(tc.tile_pool(name="persist", bufs=1))
        workp = ctx.enter_context(tc.tile_pool(name="workp", bufs=1))

        # x in FOUR batched DMAs (2 per HWDGE queue) so the first quarter
        # lands ASAP; per-DMA issue cost is ~0.6us.  Layout:
        # x_in[:, i*C + c] = x[i*128 + p, c] (t-tile-major).
        x_in = persist.tile([P, NT * C], BF16, tag="x_in", name="x_in")
        xr = x_d.ap().rearrange("(i p) c -> p i c", p=P)  # [128, 8, 512]
        xv = x_in[:].rearrange("p (i c) -> p i c", i=NT)
        nc.sync.dma_start(xv[:, 0:2, :], xr[:, 0:2, :])
        nc.scalar.dma_start(xv[:, 2:4, :], xr[:, 2:4, :])
        nc.sync.dma_start(xv[:, 4:6, :], xr[:, 4:6, :])
        nc.scalar.dma_start(xv[:, 6:8, :], xr[:, 6:8, :])

        identity = const.tile([P, P], BF16, tag="ident", name="ident")
        make_identity(nc, identity[:])
        warm_rhs = const.tile([P, CHUNK], BF16, tag="warm", name="warm_rhs")
        nc.gpsimd.memset(warm_rhs[:], 0.0)
        # ones1: K=1 all-ones lhsT for the tail's PE-broadcast normalize
        ones1 = const.tile([1, P], F32, tag="ones1", name="ones1")
        nc.gpsimd.memset(ones1[:], 1.0)

        # qkv weights: q/k columns first (they gate the qk projections),
        # v columns on the slower SWDGE queue afterwards
        wq = []  # [c-tile][128, 1536] bf16
        for m in range(NCT):
            t_ = persist.tile([P, 3 * C], BF16, tag=f"wq{m}", name=f"wq{m}")
            eng = nc.sync if m % 2 == 0 else nc.scalar
            eng.dma_start(t_[:, 0:2 * C],
                          wqkv_d.ap()[m * P:(m + 1) * P, 0:2 * C])
            wq.append(t_)
        # v columns + out-proj weights queue BEHIND the critical x/q/k
        # transfers on the same HWDGE rings (each ring drains in FIFO order,
        # so critical data gets the HBM bandwidth first)
        for m in range(NCT):
            eng = nc.sync if m % 2 == 0 else nc.scalar
            eng.dma_start(wq[m][:, 2 * C:3 * C],
                          wqkv_d.ap()[m * P:(m + 1) * P, 2 * C:3 * C])

        # bias tiles; column m = qk_b[128m:128m+128]
        qkb_all = const.tile([P, 2 * C // P], F32, tag="qkball", name="qkb_all")
        nc.gpsimd.dma_start(
            qkb_all[:], qkb_d.ap().rearrange("(m p) -> p m", p=P)
        )
        qkb_t = [qkb_all[:, m:m + 1] for m in range(2 * C // P)]
        ob_row = const.tile([1, C], F32, tag="obrow", name="ob_row")
        nc.gpsimd.dma_start(ob_row[:], ob_d.ap().rearrange("(o c) -> o c", o=1))
        ob_bcast = const.tile([P, C], F32, tag="obb", name="ob_bcast")
        nc.gpsimd.partition_broadcast(ob_bcast[:], ob_row[:], channels=P)

        # out-proj weights: single batched DMA, needed only in phase 3
        wo_all = persist.tile([P, NCT * C], BF16, tag="wo", name="wo_all")
        nc.sync.dma_start(
            wo_all[:].rearrange("p (m c) -> p m c", m=NCT),
            wout_d.ap().rearrange("(m p) c -> p m c", p=P),
        )
        wo = [wo_all[:, m * C:(m + 1) * C] for m in range(NCT)]

        # persistent activation tiles
        # xT_all[:, cc*T + t] = x^T tile cc: [c-within-tile, t]
        xT_all = persist.tile([P, NCT * T], BF16, tag="xT", name="xT_all")
        qkT = [persist.tile([P, T], BF16, tag=f"qk{m}", name=f"qk{m}")
               for m in range(NCT)]
        # kT2[m]: rows 0:64 = k^T head 2m, rows 64:128 = k^T head 2m+1
        kT2 = [persist.tile([P, T], BF16, tag=f"k2{m}", name=f"k2{m}")
               for m in range(NCT)]
        vaug = [persist.tile([P, VAW], BF16, tag=f"va{i}", name=f"va{i}")
                for i in range(NT)]
        for i in range(NT):
            nc.gpsimd.memset(vaug[i][:], 1.0)  # ones column (+padding) preset
        anorm = [persist.tile([P, T], BF16, tag=f"an{m}", name=f"an{m}")
                 for m in range(NCT)]

        # ================= prologue =================
        # HAM warm-up: real (non-transpose) matmuls on junk data while the x
        # DMA lands, so the PE clock is at 8/8 when the real work starts.
        with tc.tile_pool(name="ps_warm", bufs=1, space="PSUM") as ps_warm:
            ps_w = ps_warm.tile([P, CHUNK], F32, tag="w", name="ps_w")
            for _ in range(5):
                nc.tensor.matmul(ps_w[:], identity[:], warm_rhs[:],
                                 start=True, stop=True)

        # exp ACT-table preload: a tiny dummy exp during the DMA wait pays the
        # ~2.7us one-time table-load cost before the real pipeline needs it.
        scratch16 = workp.tile([1, 16], F32, tag="scr16", name="scratch16")
        nc.scalar.activation(
            scratch16[:], warm_rhs[0:1, 0:16],
            mybir.ActivationFunctionType.Exp, scale=1.0)



        pv_cm = tc.tile_pool(name="ps_pv", bufs=1, space="PSUM", side="right")
        state_pv_pool = pv_cm.__enter__()
        tr_cm = tc.tile_pool(name="ps_tr", bufs=2, space="PSUM", side="right")
        tr_pool = tr_cm.__enter__()

        def emit_transpose(i):
            # x PE transpose; xT_all[:, cc*T + i*128 : ...] gets tile (i, cc)
            ps_tr = tr_pool.tile([P, C], BF16, tag="tr", name="ps_tr")
            for cc in range(NCT):
                nc.tensor.transpose(
                    ps_tr[:, cc * P:(cc + 1) * P],
                    x_in[:, i * C + cc * P: i * C + (cc + 1) * P],
                    identity[:],
                )
            # one strided copy: dest [128, cc, 128] with cc-stride T
            dst = xT_all[:].rearrange("p (cc t) -> p cc t", cc=NCT)
            nc.vector.tensor_copy(
                dst[:, :, i * P:(i + 1) * P],
                ps_tr[:].rearrange("p (cc q) -> p cc q", cc=NCT),
            )

        qk_tag = [0]

        def qk_psum():
            qk_tag[0] ^= 1
            return state_pv_pool.tile(
                [P, CHUNK], F32, tag=f"pv{2 + qk_tag[0]}", name="ps_qk")

        def emit_qk(m, j):
            ps_qk = qk_psum()
            js = slice(j * CHUNK, (j + 1) * CHUNK)
            for cc in range(NCT):
                nc.tensor.matmul(
                    ps_qk[:],
                    wq[cc][:, m * P:(m + 1) * P],
                    xT_all[:, cc * T + j * CHUNK: cc * T + (j + 1) * CHUNK],
                    start=(cc == 0),
                    stop=(cc == NCT - 1),
                )
            dstt = qkT[m] if m < NCT else kT2[m - NCT]
            nc.vector.tensor_scalar_add(dstt[:, js], ps_qk[:], qkb_t[m][:])

        def emit_v(i):
            ps_v = qk_psum()
            for cc in range(NCT):
                nc.tensor.matmul(
                    ps_v[:],
                    xT_all[:, cc * T + i * P: cc * T + (i + 1) * P],
                    wq[cc][:, 2 * C:3 * C],
                    start=(cc == 0),
                    stop=(cc == NCT - 1),
                )
            va3 = vaug[i][:, 0:HEADS * VSTRIDE].rearrange(
                "p (h d) -> p h d", d=VSTRIDE)
            nc.vector.tensor_copy(
                va3[:, :, 0:HC],
                ps_v[:].rearrange("p (h d) -> p h d", h=HEADS),
            )

        # prologue part 2: transposes interleaved with pair 0's q/k tiles so
        # the first S^T slot is reachable as early as possible.  Chunk j of
        # qkT[0]/kT2[0] needs x tiles 4j..4j+3 transposed.
        for i in range(NT // 2):
            emit_transpose(i)
        emit_qk(0, 0)
        emit_qk(NCT, 0)
        for i in range(NT // 2, NT):
            emit_transpose(i)
        emit_qk(0, 1)
        emit_qk(NCT, 1)
        tr_cm.__exit__(None, None, None)

        # filler iterator: remaining phase-1 work in dependency-safe order
        def phase1_fillers():
            for m in [1, NCT + 1, 2, NCT + 2, 3, NCT + 3]:
                for j in range(NCH):
                    yield ("qk", m, j)
            for i in range(NT):
                yield ("v", i)

        fillers = phase1_fillers()
        state = {"fill_done": False}

        def run_fillers(n):
            for _ in range(n):
                try:
                    f = next(fillers)
                except StopIteration:
                    state["fill_done"] = True
                    return
                if f[0] == "qk":
                    emit_qk(f[1], f[2])
                else:
                    emit_v(f[1])

        # ================= phase 2: attention =================
        def emit_pv_chain(h, j, ppv, exh):
            base = (h % 2) * T
            for ssi in range(NT):
                nc.tensor.matmul(
                    ppv[:],
                    vaug[ssi][:, h * VSTRIDE: h * VSTRIDE + P],
                    exh[:, ssi * 2 * T + base + j * CHUNK:
                        ssi * 2 * T + base + (j + 1) * CHUNK],
                    start=(ssi == 0),
                    stop=(ssi == NT - 1),
                )

        def emit_normalize(h, ppv0, ppv1):
            m = h // 2
            rlo = (h % 2) * HC
            dtmp = workp.tile([1, T], F32, tag="dtmp", name="dtmp")
            nc.vector.tensor_copy(dtmp[:, 0:CHUNK], ppv0[HC:HC + 1, :])
            nc.vector.tensor_copy(dtmp[:, CHUNK:T], ppv1[HC:HC + 1, :])
            recip = workp.tile([1, T], F32, tag="recip", name="recip")
            nc.vector.reciprocal_approx_fast(recip[:], dtmp[:])
            bcast = workp.tile([HC, T], F32, tag="bcast", name="bcast")
            nc.gpsimd.partition_broadcast(bcast[:], recip[:], channels=HC)
            for j, ppv in ((0, ppv0), (1, ppv1)):
                nc.vector.tensor_tensor(
                    anorm[m][rlo:rlo + HC, j * CHUNK:(j + 1) * CHUNK],
                    ppv[0:HC, :],
                    bcast[:, j * CHUNK:(j + 1) * CHUNK],
                    op=mybir.AluOpType.mult,
                )

        def emit_pv_half(h, j, ppv, exh, lo):
            base = (h % 2) * T
            for ssi in range(lo, lo + NT // 2):
                nc.tensor.matmul(
                    ppv[:],
                    vaug[ssi][:, h * VSTRIDE: h * VSTRIDE + P],
                    exh[:, ssi * 2 * T + base + j * CHUNK:
                        ssi * 2 * T + base + (j + 1) * CHUNK],
                    start=(ssi == 0),
                    stop=(ssi == NT - 1),
                )

        def make_pv_steps(p, exh):
            """Ten ~0.9us payload units per pair: per head, each PV chunk
            chain is split into two 4-matmul halves, plus one normalize."""
            steps = []
            for hh in range(2):
                h = 2 * p + hh
                box = {}

                def s_alloc0(h=h, hh=hh, box=box):
                    box["p0"] = state_pv_pool.tile(
                        [P, CHUNK], F32, tag=f"pv{2 * hh}", name="ppv0")
                    emit_pv_half(h, 0, box["p0"], exh, 0)

                def s_j0b(h=h, box=box):
                    emit_pv_half(h, 0, box["p0"], exh, NT // 2)

                def s_j1a(h=h, hh=hh, box=box):
                    box["p1"] = state_pv_pool.tile(
                        [P, CHUNK], F32, tag=f"pv{2 * hh + 1}", name="ppv1")
                    emit_pv_half(h, 1, box["p1"], exh, 0)

                def s_j1b(h=h, box=box):
                    emit_pv_half(h, 1, box["p1"], exh, NT // 2)

                def s_norm(h=h, box=box):
                    emit_normalize(h, box["p0"], box["p1"])

                steps += [s_alloc0, s_j0b, s_j1a, s_j1b, s_norm]
            return steps

        exh_pool = ctx.enter_context(tc.tile_pool(name="exh", bufs=3))

        # right stack: opened after ps_tr closed, outlives the (left) qk pool
        st_cm = tc.tile_pool(name="ps_st", bufs=2, space="PSUM", side="right")
        st_pool = st_cm.__enter__()

        slots = [(p, ssi) for p in range(NPAIR) for ssi in range(NT)]
        exhs = []
        st_q = []

        def ensure_exh(p):
            while len(exhs) <= p:
                exhs.append(exh_pool.tile(
                    [P, NT * 2 * T], BF16, tag="exh", name="exh"))

        def emit_st(p, ssi):
            ensure_exh(p)
            sta = st_pool.tile([P, T], F32, tag="st", name="sta")
            stb = st_pool.tile([P, T], F32, tag="st", name="stb")
            for j in range(NCH):
                js = slice(j * CHUNK, (j + 1) * CHUNK)
                nc.tensor.matmul(
                    sta[:, js],
                    kT2[p][0:HC, ssi * P:(ssi + 1) * P],
                    qkT[p][0:HC, js],
                    start=True, stop=True,
                )
                nc.tensor.matmul(
                    stb[:, js],
                    kT2[p][HC:P, ssi * P:(ssi + 1) * P],
                    qkT[p][HC:P, js],
                    start=True, stop=True,
                )
            st_q.append((sta, stb))

        pending = []  # queue of PV/normalize closures for the previous pair
        emit_st(*slots[0])
        for g, (p, ssi) in enumerate(slots):
            exh = exhs[p]
            sta, stb = st_q.pop(0)
            nc.scalar.activation(
                exh[:, ssi * 2 * T: ssi * 2 * T + T],
                sta[:],
                mybir.ActivationFunctionType.Exp,
                scale=EXP_SCALE,
            )
            nc.scalar.activation(
                exh[:, ssi * 2 * T + T: (ssi + 1) * 2 * T],
                stb[:],
                mybir.ActivationFunctionType.Exp,
                scale=EXP_SCALE,
            )
            # next slot's S^T goes in front of this slot's payload work
            if g + 1 < len(slots):
                emit_st(*slots[g + 1])
            if ssi == 0 and p > 0:
                pending.extend(make_pv_steps(p - 1, exhs[p - 1]))
                if debug_dumps and p == 1:
                    nc.sync.dma_start(
                        dbg["dbg_exh0"].ap(), exhs[0][:, 0:2 * T])
            if ssi == 4 and p == NPAIR - 1:
                # pair 3's own PV joins the payload queue (its early-s-tile
                # halves only need already-finished exps); keeps the PE warm
                # through the last slots and shrinks the tail
                pending.extend(make_pv_steps(p, exh))
            # PE-gap payloads for this slot
            if not state["fill_done"]:
                run_fillers(2)
            if state["fill_done"]:
                npop = 2 if len(pending) >= 8 else 1
                for _ in range(npop):
                    if pending:
                        pending.pop(0)()
        # drain all remaining PV/normalize steps except pair 3's final
        # normalize, which overlaps the out-projection's cc0-2 matmuls
        while len(pending) > 1:
            pending.pop(0)()
        st_cm.__exit__(None, None, None)

        if debug_dumps:
            nc.sync.dma_start(dbg["dbg_xT"].ap(), xT_all[:])
            nc.sync.dma_start(dbg["dbg_q0"].ap(), qkT[0][:])
            nc.sync.dma_start(dbg["dbg_k0"].ap(), kT2[0][:])
            nc.sync.dma_start(dbg["dbg_va0"].ap(), vaug[0][:])

        # ================= phase 3: out projection =================
        ps_o_cm = tc.tile_pool(name="ps_o", bufs=4, space="PSUM")
        ps_op = ps_o_cm.__enter__()
        otiles = []
        for i in range(4):
            ps_o = ps_op.tile([P, C], F32, tag="o", name="ps_o")
            otiles.append(ps_o)
            for cc in range(NCT - 1):
                nc.tensor.matmul(
                    ps_o[:],
                    anorm[cc][:, i * P:(i + 1) * P],
                    wo[cc][:],
                    start=(cc == 0),
                    stop=False,
                )
        if pending:
            pending.pop(0)()  # pair 3's last normalize

        def finish_tile(i, ps_o):
            nc.tensor.matmul(
                ps_o[:],
                anorm[NCT - 1][:, i * P:(i + 1) * P],
                wo[NCT - 1][:],
                start=False,
                stop=True,
            )
            osb = workp.tile([P, C], BF16, tag=f"osb{i}", name=f"osb{i}")
            nc.vector.tensor_tensor(
                osb[:], ps_o[:], ob_bcast[:], op=mybir.AluOpType.add)
            nc.sync.dma_start(out_d.ap()[i * P:(i + 1) * P, :], osb[:])

        for i in range(4):
            finish_tile(i, otiles[i])
        for i in range(4, NT):
            ps_o = ps_op.tile([P, C], F32, tag="o", name="ps_o")
            for cc in range(NCT - 1):
                nc.tensor.matmul(
                    ps_o[:],
                    anorm[cc][:, i * P:(i + 1) * P],
                    wo[cc][:],
                    start=(cc == 0),
                    stop=False,
                )
            finish_tile(i, ps_o)
        ps_o_cm.__exit__(None, None, None)
        pv_cm.__exit__(None, None, None)
        if debug_dumps:
            nc.sync.dma_start(dbg["dbg_an0"].ap(), anorm[0][:])

    nc.compile()
    return nc


_CACHED_NC = None


def _get_nc():
    global _CACHED_NC
    if _CACHED_NC is None:
        _CACHED_NC = build_program(
            debug_dumps=bool(int(os.environ.get("KERNEL_DEBUG", "0"))))
    return _CACHED_NC


def _prep_inputs(x, qkv_w, qkv_b, out_w, out_b):
    import ml_dtypes

    x = np.asarray(x)
    B = x.shape[0]
    x2 = x.reshape(B, T, C).astype(ml_dtypes.bfloat16)
    wq2 = np.asarray(qkv_w).reshape(C, 3 * C).astype(ml_dtypes.bfloat16)
    wo2 = np.asarray(out_w).reshape(C, C).astype(ml_dtypes.bfloat16)
    qkv_b = np.asarray(qkv_b).astype(np.float32)
    out_b = np.asarray(out_b).astype(np.float32)
    # fold the v-bias through the output projection (exact: A_norm += b_v
    # shifts out by b_v @ W_out since softmax rows sum to 1).
    b_v = qkv_b[2 * C:3 * C]
    ob_eff = (
        out_b.astype(np.float64)
        + b_v.astype(np.float64) @ wo2.astype(np.float64)
    ).astype(np.float32)
    qkb = np.ascontiguousarray(qkv_b[0:2 * C])
    return x2, wq2, wo2, qkb, ob_eff


def kernel(x, qkv_w, qkv_b, out_w, out_b):
    """Full inputs in, full output out.  Shards batch across 8 NeuronCores."""
    from concourse.bass_utils import run_bass_kernel_spmd

    x = np.asarray(x)
    B, H, W, Cc = x.shape
    assert (B, H, W, Cc) == (8, 32, 32, C)
    x2, wq2, wo2, qkb, ob_eff = _prep_inputs(x, qkv_w, qkv_b, out_w, out_b)

    nc = _get_nc()
    in_maps = [
        {
            "x": np.ascontiguousarray(x2[b]),
            "qkv_w": np.ascontiguousarray(wq2),
            "out_w": np.ascontiguousarray(wo2),
            "qk_b": qkb,
            "out_b": ob_eff,
        }
        for b in range(B)
    ]
    trace = bool(int(os.environ.get("KERNEL_TRACE", "0")))
    res = run_bass_kernel_spmd(nc, in_maps, core_ids=list(range(B)), trace=trace)
    if trace and res.exec_time_ns is not None:
        print(f"HW exec time: {res.exec_time_ns} ns")
    kernel.last_results = res
    out = np.stack(
        [np.asarray(res.results[b]["out"]).astype(np.float32) for b in range(B)],
        axis=0,
    )
    return out.reshape(B, H, W, Cc)


kernel.last_results = None


# revision 47
# speedup vs baseline: 1.4001x; 1.0240x over previous
"""Trainium2 Bass kernel for nn_AttentionBlock (B=8, H=W=32, C=512, 8 heads).

Strategy: data-parallel over batch -- each of the 8 NeuronCores processes one
batch element end-to-end (no collectives).  Per core:

  x [T=1024, C=512] -> qkv -> per-head attention (T x T softmax) -> out proj.

v2 design (ACT-bound pipeline):
  * all matmul data is bf16 (host pre-casts); PSUM accumulation stays fp32.
  * S^T = k^T q is computed with K=64 ROW-TILED matmul pairs: head 2p on PE
    row-groups 0-1 (SBUF partitions 0:64), head 2p+1 on row-groups 2-3
    (partitions 64:128).  The two matmuls run concurrently in the array, so
    a head-pair s-tile costs ~2x512 columns instead of 4x512.
  * softmax exp runs on ScalarE (the only exp engine) in N=1024 calls and is
    the phase-2 bottleneck (~73us); everything else (QKV projection, PV,
    out-projection) is woven into the PE gaps between exp calls.
  * denominators come free from a ones-column appended to V (PV row 64);
    normalization = DVE multiply by a GPSIMD-broadcast reciprocal.
  * out-projection uses anorm tiles as lhsT so the output lands directly in
    [t, c] layout -- no output transpose.  Output is bf16; host casts back.
  * no max-subtraction: logits are ~N(0,1) by construction (1/8 scale is
    folded into the ScalarE exp activation).
"""

import math
import os
from contextlib import ExitStack

import numpy as np

import concourse.bass as bass
import concourse.mybir as mybir
import concourse.tile as tile
from concourse import bacc

T = 1024          # tokens per batch element (32*32)
C = 512           # channels
HEADS = 8
HC = C // HEADS   # 64
P = 128           # partitions
NT = T // P       # 8 t-tiles (also 8 s-tiles)
NCT = C // P      # 4 c-tiles
CHUNK = 512       # matmul moving-operand chunk (one fp32 PSUM bank)
NCH = T // CHUNK  # 2 chunks
NPAIR = HEADS // 2
F32 = mybir.dt.float32
BF16 = mybir.dt.bfloat16
EXP_SCALE = 1.0 / math.sqrt(HC)  # (1/sqrt(sqrt(hc)))^2 applied to q.k
VSTRIDE = HC + 1  # 65: v columns + ones column per head
VAW = HEADS * VSTRIDE + (P - VSTRIDE)  # PV lhsT 128-wide reads stay in-tile


def build_program(debug_dumps=False):
    nc = bacc.Bacc("TRN2", num_devices=8, debug=False)

    x_d = nc.dram_tensor("x", [T, C], BF16, kind="ExternalInput")
    wqkv_d = nc.dram_tensor("qkv_w", [C, 3 * C], BF16, kind="ExternalInput")
    wout_d = nc.dram_tensor("out_w", [C, C], BF16, kind="ExternalInput")
    qkb_d = nc.dram_tensor("qk_b", [2 * C], F32, kind="ExternalInput")
    ob_d = nc.dram_tensor("out_b", [C], F32, kind="ExternalInput")
    out_d = nc.dram_tensor("out", [T, C], BF16, kind="ExternalOutput")
    dbg = {}
    if debug_dumps:
        for nm, shp in [
            ("dbg_xT", [P, NCT * T]), ("dbg_q0", [P, T]), ("dbg_k0", [P, T]),
            ("dbg_exh0", [P, 2 * T]), ("dbg_va0", [P, VAW]),
            ("dbg_an0", [P, T]),
        ]:
            dbg[nm] = nc.dram_tensor(nm, shp, BF16, kind="ExternalOutput")

    with tile.TileContext(nc) as tc, ExitStack() as ctx:
        from concourse.masks import make_identity

        # ---------------- SBUF pools ----------------
        const = ctx.enter_context(tc.tile_pool(name="const", bufs=1))
        persist = ctx.enter_context(tc.tile_pool(name="persist", bufs=1))
        workp = ctx.enter_context(tc.tile_pool(name="workp", bufs=1))

        # x in FOUR batched DMAs (2 per HWDGE queue) so the first quarter
        # lands ASAP; per-DMA issue cost is ~0.6us.  Layout:
        # x_in[:, i*C + c] = x[i*128 + p, c] (t-tile-major).
        x_in = persist.tile([P, NT * C], BF16, tag="x_in", name="x_in")
        xr = x_d.ap().rearrange("(i p) c -> p i c", p=P)  # [128, 8, 512]
        xv = x_in[:].rearrange("p (i c) -> p i c", i=NT)
        nc.sync.dma_start(xv[:, 0:2, :], xr[:, 0:2, :])
        nc.scalar.dma_start(xv[:, 2:4, :], xr[:, 2:4, :])
        nc.sync.dma_start(xv[:, 4:6, :], xr[:, 4:6, :])
        nc.scalar.dma_start(xv[:, 6:8, :], xr[:, 6:8, :])

        identity = const.tile([P, P], BF16, tag="ident", name="ident")
        make_identity(nc, identity[:])
        warm_rhs = const.tile([P, CHUNK], BF16, tag="warm", name="warm_rhs")
        nc.gpsimd.memset(warm_rhs[:], 0.0)
        # ones1: K=1 all-ones lhsT for the tail's PE-broadcast normalize
        ones1 = const.tile([1, P], F32, tag="ones1", name="ones1")
        nc.gpsimd.memset(ones1[:], 1.0)

        # qkv weights: q/k columns first (they gate the qk projections),
        # v columns on the slower SWDGE queue afterwards
        wq = []  # [c-tile][128, 1536] bf16
        for m in range(NCT):
            t_ = persist.tile([P, 3 * C], BF16, tag=f"wq{m}", name=f"wq{m}")
            eng = nc.sync if m % 2 == 0 else nc.scalar
            eng.dma_start(t_[:, 0:2 * C],
                          wqkv_d.ap()[m * P:(m + 1) * P, 0:2 * C])
            wq.append(t_)
        # v columns + out-proj weights queue BEHIND the critical x/q/k
        # transfers on the same HWDGE rings (each ring drains in FIFO order,
        # so critical data gets the HBM bandwidth first)
        for m in range(NCT):
            eng = nc.sync if m % 2 == 0 else nc.scalar
            eng.dma_start(wq[m][:, 2 * C:3 * C],
                          wqkv_d.ap()[m * P:(m + 1) * P, 2 * C:3 * C])

        # bias tiles; column m = qk_b[128m:128m+128]
        qkb_all = const.tile([P, 2 * C // P], F32, tag="qkball", name="qkb_all")
        nc.gpsimd.dma_start(
            qkb_all[:], qkb_d.ap().rearrange("(m p) -> p m", p=P)
        )
        qkb_t = [qkb_all[:, m:m + 1] for m in range(2 * C // P)]
        ob_row = const.tile([1, C], F32, tag="obrow", name="ob_row")
        nc.gpsimd.dma_start(ob_row[:], ob_d.ap().rearrange("(o c) -> o c", o=1))
        ob_bcast = const.tile([P, C], F32, tag="obb", name="ob_bcast")
        nc.gpsimd.partition_broadcast(ob_bcast[:], ob_row[:], channels=P)

        # out-proj weights: single batched DMA, needed only in phase 3
        wo_all = persist.tile([P, NCT * C], BF16, tag="wo", name="wo_all")
        nc.sync.dma_start(
            wo_all[:].rearrange("p (m c) -> p m c", m=NCT),
            wout_d.ap().rearrange("(m p) c -> p m c", p=P),
        )
        wo = [wo_all[:, m * C:(m + 1) * C] for m in range(NCT)]

        # persistent activation tiles
        # xT_all[:, cc*T + t] = x^T tile cc: [c-within-tile, t]
        xT_all = persist.tile([P, NCT * T], BF16, tag="xT", name="xT_all")
        qkT = [persist.tile([P, T], BF16, tag=f"qk{m}", name=f"qk{m}")
               for m in range(NCT)]
        # kT2[m]: rows 0:64 = k^T head 2m, rows 64:128 = k^T head 2m+1
        kT2 = [persist.tile([P, T], BF16, tag=f"k2{m}", name=f"k2{m}")
               for m in range(NCT)]
        vaug = [persist.tile([P, VAW], BF16, tag=f"va{i}", name=f"va{i}")
                for i in range(NT)]
        for i in range(NT):
            nc.gpsimd.memset(vaug[i][:], 1.0)  # ones column (+padding) preset
        anorm = [persist.tile([P, T], BF16, tag=f"an{m}", name=f"an{m}")
                 for m in range(NCT)]

        # ================= prologue =================
        # HAM warm-up: real (non-transpose) matmuls on junk data while the x
        # DMA lands, so the PE clock is at 8/8 when the real work starts.
        with tc.tile_pool(name="ps_warm", bufs=1, space="PSUM") as ps_warm:
            ps_w = ps_warm.tile([P, CHUNK], F32, tag="w", name="ps_w")
            for _ in range(5):
                nc.tensor.matmul(ps_w[:], identity[:], warm_rhs[:],
                                 start=True, stop=True)

        # exp ACT-table preload: a tiny dummy exp during the DMA wait pays the
        # ~2.7us one-time table-load cost before the real pipeline needs it.
        scratch16 = workp.tile([1, 16], F32, tag="scr16", name="scratch16")
        nc.scalar.activation(
            scratch16[:], warm_rhs[0:1, 0:16],
            mybir.ActivationFunctionType.Exp, scale=1.0)



        pv_cm = tc.tile_pool(name="ps_pv", bufs=1, space="PSUM", side="right")
        state_pv_pool = pv_cm.__enter__()
        tr_cm = tc.tile_pool(name="ps_tr", bufs=2, space="PSUM", side="right")
        tr_pool = tr_cm.__enter__()

        def emit_transpose(i):
            # x PE transpose; xT_all[:, cc*T + i*128 : ...] gets tile (i, cc)
            ps_tr = tr_pool.tile([P, C], BF16, tag="tr", name="ps_tr")
            for cc in range(NCT):
                nc.tensor.transpose(
                    ps_tr[:, cc * P:(cc + 1) * P],
                    x_in[:, i * C + cc * P: i * C + (cc + 1) * P],
                    identity[:],
                )
            # one strided copy: dest [128, cc, 128] with cc-stride T
            dst = xT_all[:].rearrange("p (cc t) -> p cc t", cc=NCT)
            nc.vector.tensor_copy(
                dst[:, :, i * P:(i + 1) * P],
                ps_tr[:].rearrange("p (cc q) -> p cc q", cc=NCT),
            )

        qk_tag = [0]

        def qk_psum():
            qk_tag[0] ^= 1
            return state_pv_pool.tile(
                [P, CHUNK], F32, tag=f"pv{2 + qk_tag[0]}", name="ps_qk")

        def emit_qk(m, j):
            ps_qk = qk_psum()
            js = slice(j * CHUNK, (j + 1) * CHUNK)
            for cc in range(NCT):
                nc.tensor.matmul(
                    ps_qk[:],
                    wq[cc][:, m * P:(m + 1) * P],
                    xT_all[:, cc * T + j * CHUNK: cc * T + (j + 1) * CHUNK],
                    start=(cc == 0),
                    stop=(cc == NCT - 1),
                )
            dstt = qkT[m] if m < NCT else kT2[m - NCT]
            nc.vector.tensor_scalar_add(dstt[:, js], ps_qk[:], qkb_t[m][:])

        def emit_v(i):
            ps_v = qk_psum()
            for cc in range(NCT):
                nc.tensor.matmul(
                    ps_v[:],
                    xT_all[:, cc * T + i * P: cc * T + (i + 1) * P],
                    wq[cc][:, 2 * C:3 * C],
                    start=(cc == 0),
                    stop=(cc == NCT - 1),
                )
            va3 = vaug[i][:, 0:HEADS * VSTRIDE].rearrange(
                "p (h d) -> p h d", d=VSTRIDE)
            nc.vector.tensor_copy(
                va3[:, :, 0:HC],
                ps_v[:].rearrange("p (h d) -> p h d", h=HEADS),
            )

        # prologue part 2: transposes interleaved with pair 0's q/k tiles so
        # the first S^T slot is reachable as early as possible.  Chunk j of
        # qkT[0]/kT2[0] needs x tiles 4j..4j+3 transposed.
        for i in range(NT // 2):
            emit_transpose(i)
        emit_qk(0, 0)
        emit_qk(NCT, 0)
        for i in range(NT // 2, NT):
            emit_transpose(i)
        emit_qk(0, 1)
        emit_qk(NCT, 1)
        tr_cm.__exit__(None, None, None)

        # filler iterator: remaining phase-1 work in dependency-safe order
        def phase1_fillers():
            for m in [1, NCT + 1, 2, NCT + 2, 3, NCT + 3]:
                for j in range(NCH):
                    yield ("qk", m, j)
            for i in range(NT):
                yield ("v", i)

        fillers = phase1_fillers()
        state = {"fill_done": False}

        def run_fillers(n):
            for _ in range(n):
                try:
                    f = next(fillers)
                except StopIteration:
                    state["fill_done"] = True
                    return
                if f[0] == "qk":
                    emit_qk(f[1], f[2])
                else:
                    emit_v(f[1])

        # ================= phase 2: attention =================
        def emit_pv_chain(h, j, ppv, exh):
            base = (h % 2) * T
            for ssi in range(NT):
                nc.tensor.matmul(
                    ppv[:],
                    vaug[ssi][:, h * VSTRIDE: h * VSTRIDE + P],
                    exh[:, ssi * 2 * T + base + j * CHUNK:
                        ssi * 2 * T + base + (j + 1) * CHUNK],
                    start=(ssi == 0),
                    stop=(ssi == NT - 1),
                )

        def emit_recip(h, box):
            hh = h % 2
            dtmp = workp.tile([1, T], F32, tag=f"dtmp{hh}", name="dtmp")
            nc.vector.tensor_copy(dtmp[:, 0:CHUNK], box["p0"][HC:HC + 1, :])
            nc.vector.tensor_copy(dtmp[:, CHUNK:T], box["p1"][HC:HC + 1, :])
            recip = workp.tile([1, T], F32, tag=f"recip{hh}", name="recip")
            nc.vector.reciprocal_approx_fast(recip[:], dtmp[:])
            bcast = workp.tile([HC, T], F32, tag=f"bcast{hh}", name="bcast")
            nc.gpsimd.partition_broadcast(bcast[:], recip[:], channels=HC)
            box["bc"] = bcast

        def emit_mults(h, box):
            m = h // 2
            rlo = (h % 2) * HC
            for j, ppv in ((0, box["p0"]), (1, box["p1"])):
                nc.vector.tensor_tensor(
                    anorm[m][rlo:rlo + HC, j * CHUNK:(j + 1) * CHUNK],
                    ppv[0:HC, :],
                    box["bc"][:, j * CHUNK:(j + 1) * CHUNK],
                    op=mybir.AluOpType.mult,
                )

        def emit_pv_half(h, j, ppv, exh, lo):
            base = (h % 2) * T
            for ssi in range(lo, lo + NT // 2):
                nc.tensor.matmul(
                    ppv[:],
                    vaug[ssi][:, h * VSTRIDE: h * VSTRIDE + P],
                    exh[:, ssi * 2 * T + base + j * CHUNK:
                        ssi * 2 * T + base + (j + 1) * CHUNK],
                    start=(ssi == 0),
                    stop=(ssi == NT - 1),
                )

        def make_pv_steps(p, exh):
            """Twelve ~0.9us payload units per pair: per head, each PV chunk
            chain is split into two 4-matmul halves plus a reciprocal step;
            both heads' normalize-multiplies come last (recips overlap)."""
            steps = []
            tail_mults = []
            for hh in range(2):
                h = 2 * p + hh
                box = {}

                def s_alloc0(h=h, hh=hh, box=box):
                    box["p0"] = state_pv_pool.tile(
                        [P, CHUNK], F32, tag=f"pv{2 * hh}", name="ppv0")
                    emit_pv_half(h, 0, box["p0"], exh, 0)

                def s_j0b(h=h, box=box):
                    emit_pv_half(h, 0, box["p0"], exh, NT // 2)

                def s_j1a(h=h, hh=hh, box=box):
                    box["p1"] = state_pv_pool.tile(
                        [P, CHUNK], F32, tag=f"pv{2 * hh + 1}", name="ppv1")
                    emit_pv_half(h, 1, box["p1"], exh, 0)

                def s_j1b(h=h, box=box):
                    emit_pv_half(h, 1, box["p1"], exh, NT // 2)

                def s_recip(h=h, box=box):
                    emit_recip(h, box)

                def s_mults(h=h, box=box):
                    emit_mults(h, box)

                steps += [s_alloc0, s_j0b, s_j1a, s_j1b, s_recip]
                tail_mults.append(s_mults)
            steps += tail_mults[-2:]
            del tail_mults[-2:]
            return steps

        exh_pool = ctx.enter_context(tc.tile_pool(name="exh", bufs=3))

        # right stack: opened after ps_tr closed, outlives the (left) qk pool
        st_cm = tc.tile_pool(name="ps_st", bufs=2, space="PSUM", side="right")
        st_pool = st_cm.__enter__()

        slots = [(p, ssi) for p in range(NPAIR) for ssi in range(NT)]
        exhs = []
        st_q = []

        def ensure_exh(p):
            while len(exhs) <= p:
                exhs.append(exh_pool.tile(
                    [P, NT * 2 * T], BF16, tag="exh", name="exh"))

        def emit_st(p, ssi):
            ensure_exh(p)
            sta = st_pool.tile([P, T], F32, tag="st", name="sta")
            stb = st_pool.tile([P, T], F32, tag="st", name="stb")
            for j in range(NCH):
                js = slice(j * CHUNK, (j + 1) * CHUNK)
                nc.tensor.matmul(
                    sta[:, js],
                    kT2[p][0:HC, ssi * P:(ssi + 1) * P],
                    qkT[p][0:HC, js],
                    start=True, stop=True,
                )
                nc.tensor.matmul(
                    stb[:, js],
                    kT2[p][HC:P, ssi * P:(ssi + 1) * P],
                    qkT[p][HC:P, js],
                    start=True, stop=True,
                )
            st_q.append((sta, stb))

        pending = []  # queue of PV/normalize closures for the previous pair
        emit_st(*slots[0])
        for g, (p, ssi) in enumerate(slots):
            exh = exhs[p]
            sta, stb = st_q.pop(0)
            nc.scalar.activation(
                exh[:, ssi * 2 * T: ssi * 2 * T + T],
                sta[:],
                mybir.ActivationFunctionType.Exp,
                scale=EXP_SCALE,
            )
            nc.scalar.activation(
                exh[:, ssi * 2 * T + T: (ssi + 1) * 2 * T],
                stb[:],
                mybir.ActivationFunctionType.Exp,
                scale=EXP_SCALE,
            )
            # next slot's S^T goes in front of this slot's payload work
            if g + 1 < len(slots):
                emit_st(*slots[g + 1])
            if ssi == 0 and p > 0:
                pending.extend(make_pv_steps(p - 1, exhs[p - 1]))
                if debug_dumps and p == 1:
                    nc.sync.dma_start(
                        dbg["dbg_exh0"].ap(), exhs[0][:, 0:2 * T])
            if ssi == 4 and p == NPAIR - 1:
                # pair 3's own PV joins the payload queue (its early-s-tile
                # halves only need already-finished exps); keeps the PE warm
                # through the last slots and shrinks the tail
                pending.extend(make_pv_steps(p, exh))
            # PE-gap payloads for this slot
            if not state["fill_done"]:
                run_fillers(2)
            if state["fill_done"]:
                npop = 2 if len(pending) >= 6 else 1
                for _ in range(npop):
                    if pending:
                        pending.pop(0)()
        # drain all remaining PV/normalize steps except pair 3's final
        # normalize-multiplies, which overlap the out-projection's cc0-2 work
        while len(pending) > 2:
            pending.pop(0)()
        st_cm.__exit__(None, None, None)

        if debug_dumps:
            nc.sync.dma_start(dbg["dbg_xT"].ap(), xT_all[:])
            nc.sync.dma_start(dbg["dbg_q0"].ap(), qkT[0][:])
            nc.sync.dma_start(dbg["dbg_k0"].ap(), kT2[0][:])
            nc.sync.dma_start(dbg["dbg_va0"].ap(), vaug[0][:])

        # ================= phase 3: out projection =================
        ps_o_cm = tc.tile_pool(name="ps_o", bufs=3, space="PSUM")
        ps_op = ps_o_cm.__enter__()
        otiles = []
        for i in range(3):
            ps_o = ps_op.tile([P, C], F32, tag="o", name="ps_o")
            otiles.append(ps_o)
            for cc in range(NCT - 1):
                nc.tensor.matmul(
                    ps_o[:],
                    anorm[cc][:, i * P:(i + 1) * P],
                    wo[cc][:],
                    start=(cc == 0),
                    stop=False,
                )
        while pending:
            pending.pop(0)()  # pair 3's normalize-multiplies

        def finish_tile(i, ps_o):
            nc.tensor.matmul(
                ps_o[:],
                anorm[NCT - 1][:, i * P:(i + 1) * P],
                wo[NCT - 1][:],
                start=False,
                stop=True,
            )
            osb = workp.tile([P, C], BF16, tag=f"osb{i}", name=f"osb{i}")
            nc.vector.tensor_tensor(
                osb[:], ps_o[:], ob_bcast[:], op=mybir.AluOpType.add)
            nc.sync.dma_start(out_d.ap()[i * P:(i + 1) * P, :], osb[:])

        for i in range(3):
            finish_tile(i, otiles[i])
        for i in range(3, NT):
            ps_o = ps_op.tile([P, C], F32, tag="o", name="ps_o")
            for cc in range(NCT - 1):
                nc.tensor.matmul(
                    ps_o[:],
                    anorm[cc][:, i * P:(i + 1) * P],
                    wo[cc][:],
                    start=(cc == 0),
                    stop=False,
                )
            finish_tile(i, ps_o)
        ps_o_cm.__exit__(None, None, None)
        pv_cm.__exit__(None, None, None)
        if debug_dumps:
            nc.sync.dma_start(dbg["dbg_an0"].ap(), anorm[0][:])

    nc.compile()
    return nc


_CACHED_NC = None


def _get_nc():
    global _CACHED_NC
    if _CACHED_NC is None:
        _CACHED_NC = build_program(
            debug_dumps=bool(int(os.environ.get("KERNEL_DEBUG", "0"))))
    return _CACHED_NC


def _prep_inputs(x, qkv_w, qkv_b, out_w, out_b):
    import ml_dtypes

    x = np.asarray(x)
    B = x.shape[0]
    x2 = x.reshape(B, T, C).astype(ml_dtypes.bfloat16)
    wq2 = np.asarray(qkv_w).reshape(C, 3 * C).astype(ml_dtypes.bfloat16)
    wo2 = np.asarray(out_w).reshape(C, C).astype(ml_dtypes.bfloat16)
    qkv_b = np.asarray(qkv_b).astype(np.float32)
    out_b = np.asarray(out_b).astype(np.float32)
    # fold the v-bias through the output projection (exact: A_norm += b_v
    # shifts out by b_v @ W_out since softmax rows sum to 1).
    b_v = qkv_b[2 * C:3 * C]
    ob_eff = (
        out_b.astype(np.float64)
        + b_v.astype(np.float64) @ wo2.astype(np.float64)
    ).astype(np.float32)
    qkb = np.ascontiguousarray(qkv_b[0:2 * C])
    return x2, wq2, wo2, qkb, ob_eff


def kernel(x, qkv_w, qkv_b, out_w, out_b):
    """Full inputs in, full output out.  Shards batch across 8 NeuronCores."""
    from concourse.bass_utils import run_bass_kernel_spmd

    x = np.asarray(x)
    B, H, W, Cc = x.shape
    assert (B, H, W, Cc) == (8, 32, 32, C)
    x2, wq2, wo2, qkb, ob_eff = _prep_inputs(x, qkv_w, qkv_b, out_w, out_b)

    nc = _get_nc()
    in_maps = [
        {
            "x": np.ascontiguousarray(x2[b]),
            "qkv_w": np.ascontiguousarray(wq2),
            "out_w": np.ascontiguousarray(wo2),
            "qk_b": qkb,
            "out_b": ob_eff,
        }
        for b in range(B)
    ]
    trace = bool(int(os.environ.get("KERNEL_TRACE", "0")))
    res = run_bass_kernel_spmd(nc, in_maps, core_ids=list(range(B)), trace=trace)
    if trace and res.exec_time_ns is not None:
        print(f"HW exec time: {res.exec_time_ns} ns")
    kernel.last_results = res
    out = np.stack(
        [np.asarray(res.results[b]["out"]).astype(np.float32) for b in range(B)],
        axis=0,
    )
    return out.reshape(B, H, W, Cc)


kernel.last_results = None
